# revision 23
# baseline (speedup 1.0000x reference)
"""Multi-head self-attention (causal) Trainium2 Bass/Tile kernel, 8-way SPMD.

Sharding: data-parallel over batch (4) x tensor-parallel over heads (2 groups
of 8 heads).  Core c handles batch c//2, head-group c%2.  Each core computes
q/k/v projections for its 512 local features, causal attention for its 8
heads, and a partial o-projection (contraction over its 512 features of the
attention output) giving a full-shape [S, D] partial that the host sums per
batch pair.

All matmul operands are bf16 (fp32 PSUM accumulation); softmax runs without
max-subtraction (scores ~ N(0,1) after the 1/8 scale, no overflow risk), with
exp on the scalar engine and the row-sum folded into the AV matmul via a ones
column appended to V.  Host pre-transposes inputs so no on-chip transposes
are needed:
  qT[e,s]  = wqT.T @ xT        (lhsT=wqT[d,e], rhs=xT[d,s])
  scoresT[sk,sq] = kT.T @ qT   (lhsT=kT[dk,sk], rhs=qT[dk,sq], K=64)
  avT[dk+1,sq]   = vaug.T @ expT  (lhsT=vaug[sk,65], rhs=expT[sk,sq])
  y[s,e]   = outT.T @ woT      (lhsT=outT[d,s], rhs=woT[d,e])

v3 vs v2 (307.5us):
 - masking back on DVE (PE tri-add cost more PE than it saved), but only on
   the trimmed 128-wide diagonal sub-block ([128,2,128] strided mul, ~194ns).
 - filler projection/oproj units are injected INSIDE each attention kt loop
   at spread points, so the PE never stalls on the exp stream's PSUM-buffer
   recycling (the v2 per-slot ~1.1-1.4us gaps).
 - query groups visit in order [0,1,3,2]: the ACT-heavy qg3 slots run while
   oproj filler still exists; qg2 (last) uses per-pair normalization and the
   final slot normalizes inline.
 - warmup back to 16 x N=512 matmuls (N=128 failed to trip the HAM window).
"""

from contextlib import ExitStack

import numpy as np
import ml_dtypes

import concourse.bass as bass
import concourse.tile as tile
from concourse import bacc, mybir
from concourse._compat import with_exitstack
from concourse.bass_utils import run_bass_kernel_spmd

B, S, D, H = 4, 2048, 1024, 16
DK = D // H          # 64
E = 512              # local features per core (8 heads)
HL = 8               # local heads
NCORES = 8
NDT = D // 128       # 8 d-tiles
NET = E // 128       # 4 e-tiles
NST = S // 128       # 16 s-tiles
NQG = S // 512       # 4 query groups

QG_ORDER = [0, 1, 3, 2]          # visit order; last visited gets inline norm
LAST_QG = QG_ORDER[-1]

F32 = mybir.dt.float32
BF16 = mybir.dt.bfloat16
bf16 = ml_dtypes.bfloat16

_compiled = None
last_results = None  # test harness introspection


@with_exitstack
def _mhsa_kernel(ctx: ExitStack, tc: tile.TileContext, y, xT, wqT, wkT, wvT,
                 woT, m01):
    nc = tc.nc

    consts = ctx.enter_context(tc.tile_pool(name="consts", bufs=1))
    ex_pool = ctx.enter_context(tc.tile_pool(name="ex", bufs=8))
    rec_pool = ctx.enter_context(tc.tile_pool(name="rec", bufs=2))
    y_pool = ctx.enter_context(tc.tile_pool(name="ysb", bufs=3))
    ps_pool = ctx.enter_context(tc.tile_pool(name="psmm", bufs=2, space="PSUM"))
    fl_pool = ctx.enter_context(tc.tile_pool(name="psfl", bufs=2, space="PSUM"))
    av_pool = ctx.enter_context(tc.tile_pool(name="psav", bufs=2, space="PSUM"))

    def ctile(shape, dt_, tg):
        return consts.tile(shape, dt_, tag=tg, name=tg)

    # ---- persistent SBUF tiles -------------------------------------------
    # x and weights live in single wide tiles (one batched strided DMA per
    # tensor; ~40 small descriptors serialized at ~600ns each on two queues
    # was stretching the input load to ~29us).  The per-d-tile names below
    # are views, so downstream indexing is unchanged.
    x_all = ctile([128, NDT * S], BF16, "xall")
    wq_all = ctile([128, NDT * E], BF16, "wqall")
    wk_all = ctile([128, NDT * E], BF16, "wkall")
    wv_all = ctile([128, NDT * E], BF16, "wvall")
    wo_all = ctile([128, NET * D], BF16, "woall")
    xT_t = [x_all[:, i * S:(i + 1) * S] for i in range(NDT)]
    wqT_t = [wq_all[:, i * E:(i + 1) * E] for i in range(NDT)]
    wkT_t = [wk_all[:, i * E:(i + 1) * E] for i in range(NDT)]
    wvT_t = [wv_all[:, i * E:(i + 1) * E] for i in range(NDT)]
    woT_t = [wo_all[:, i * D:(i + 1) * D] for i in range(NET)]
    qT_t = [ctile([128, S], BF16, f"qT{i}") for i in range(NET)]
    kT_t = [ctile([128, S], BF16, f"kT{i}") for i in range(NET)]
    vaug_t = [ctile([128, HL * (DK + 1)], BF16, f"vaug{i}") for i in range(NST)]
    outT_t = [ctile([128, S], BF16, f"outT{i}") for i in range(NET)]
    m01_t = ctile([128, 256], BF16, "m01")

    # ---- input loads, ordered so compute can start ASAP ------------------
    for c in range(4):
        nc.gpsimd.dma_start(
            out=x_all.rearrange("p (i s) -> p i s", s=S)[:, :,
                                                         c * 512:(c + 1) * 512],
            in_=xT.rearrange("(i p) s -> p i s", p=128)[:, :,
                                                        c * 512:(c + 1) * 512])
    nc.sync.dma_start(out=wv_all.rearrange("p (i e) -> p i e", e=E),
                      in_=wvT.rearrange("(i p) e -> p i e", p=128))
    nc.sync.dma_start(out=m01_t, in_=m01)
    nc.sync.dma_start(out=wq_all.rearrange("p (i e) -> p i e", e=E),
                      in_=wqT.rearrange("(i p) e -> p i e", p=128))
    nc.sync.dma_start(out=wk_all.rearrange("p (i e) -> p i e", e=E),
                      in_=wkT.rearrange("(i p) e -> p i e", p=128))
    nc.sync.dma_start(out=wo_all.rearrange("p (i e) -> p i e", e=D),
                      in_=woT.rearrange("(i p) e -> p i e", p=128))

    # ---- q/k projections: qT[e,s], kT[e,s] -------------------------------
    # Fillers use their own [128,512] PSUM pool so a filler matmul never
    # waits on a score tile's exp or another filler's DVE drain.
    def proj_qk_h(wt, dst, et, scg, hf):
        ps = fl_pool.tile([128, 512], F32, tag="fl", name="fps")
        s0 = scg * 1024 + hf * 512
        for dt_ in range(NDT):
            nc.tensor.matmul(
                ps,
                lhsT=wt[dt_][:, et * 128:(et + 1) * 128],
                rhs=xT_t[dt_][:, s0:s0 + 512],
                start=(dt_ == 0), stop=(dt_ == NDT - 1),
            )
        nc.vector.tensor_copy(dst[et][:, s0:s0 + 512], ps)

    def proj_qk(wt, dst, et, scg):
        proj_qk_h(wt, dst, et, scg, 0)
        proj_qk_h(wt, dst, et, scg, 1)

    # ---- v projection -> vaug tiles [128, 8*65] with ones columns --------
    def proj_v_st(st):
        ps = fl_pool.tile([128, 512], F32, tag="fl", name="fps")
        for dt_ in range(NDT):
            nc.tensor.matmul(
                ps,
                lhsT=xT_t[dt_][:, st * 128:(st + 1) * 128],
                rhs=wvT_t[dt_],
                start=(dt_ == 0), stop=(dt_ == NDT - 1),
            )
        nc.vector.memset(vaug_t[st], 1.0)
        nc.vector.tensor_copy(
            vaug_t[st].rearrange("p (h c) -> p h c", c=65)[:, :, 0:64],
            ps.rearrange("p (h c) -> p h c", c=64),
        )

    def proj_v(stp):
        proj_v_st(2 * stp)
        proj_v_st(2 * stp + 1)

    sums_dram = nc.dram_tensor("sums_bounce", [NQG, HL, 512], F32).ap()
    rec_dram = nc.dram_tensor("rec_bounce", [NQG, HL, 512], BF16).ap()

    # ones2: selector for the final pair's reciprocal broadcast matmul
    ones2 = ctile([64, 128], BF16, "ones2")
    nc.vector.memset(ones2, 0.0)
    nc.vector.memset(ones2[0:1, 0:64], 1.0)
    nc.vector.memset(ones2[32:33, 64:128], 1.0)


    # PE warm-up: ~6.8us of solid matmul streaming releases the HAM throttle
    warm = ctile([128, 512], BF16, "warm")
    nc.vector.memset(warm, 0.0)
    for _ in range(16):
        wps = ps_pool.tile([128, 512], F32, tag="mm", name="wps")
        nc.tensor.matmul(wps, lhsT=warm[:, 0:128], rhs=warm,
                         start=True, stop=True)

    # ---- attention for one (head-pair, query-group) ----------------------
    # AV runs as a col-tiled concurrent pair: head A -> av2[0:64] (array col
    # groups 0-1), head B -> av2[64:128] (groups 2-3); one ~225ns pass per
    # kt instead of two serial M=65 passes.  Softmax denominators become 4
    # concurrent M=1 col-tiled matmuls per kt PAIR (ones.T @ ex), landing at
    # den rows 0 (A,even kt), 32 (B,even), 64 (A,odd), 96 (B,odd); each
    # stream PSUM-accumulates across its kts.  The stash recombines
    # even+odd partials with one shifted copy + one add.
    def attn(hp, qg, units):
        ti = hp
        hA, hB = 2 * hp, 2 * hp + 1
        nk = 4 * qg + 4
        avA = av_pool.tile([65, 512], F32, tag="av", name="avA")
        avB = av_pool.tile([65, 512], F32, tag="av", name="avB")

        n_u = len(units)
        inject = {}
        for i in range(n_u):
            pt = (i + 1) * (nk - 1) // n_u if n_u else 0
            inject.setdefault(pt, []).append(units[i])

        def pop_one():
            kt, ex, off = pending.pop(0)
            for av, h in ((avA, hA), (avB, hB)):
                nc.tensor.matmul(
                    av[:, off:512],
                    lhsT=vaug_t[kt][:, h * 65:h * 65 + 65],
                    rhs=ex[:, (h & 1) * 512 + off:((h & 1) + 1) * 512],
                    start=(kt == 0), stop=(kt == nk - 1),
                    skip_group_check=True,
                )

        pending = []

        for kt in range(nk):
            j = kt - 4 * qg
            off = 128 * j if j >= 0 else 0
            diag = j >= 0
            ps = ps_pool.tile([128, 1024], F32, tag="mm", name="ps")
            for po in (0, 64):
                hf = po // 64
                nc.tensor.matmul(
                    ps[:, hf * 512 + off:(hf + 1) * 512],
                    lhsT=kT_t[ti][po:po + 64, kt * 128:(kt + 1) * 128],
                    rhs=qT_t[ti][po:po + 64, qg * 512 + off:(qg + 1) * 512],
                    start=True, stop=True,
                )
            ex = ex_pool.tile([128, 1024], BF16, tag="ex", name="ex")
            if off:
                ps_in = ps.rearrange("p (h q) -> p h q", q=512)[:, :, off:512]
                ex_out = ex.rearrange("p (h q) -> p h q", q=512)[:, :, off:512]
            else:
                ps_in, ex_out = ps, ex
            nc.scalar.activation(out=ex_out, in_=ps_in,
                                 func=mybir.ActivationFunctionType.Exp,
                                 scale=0.125)
            if diag:  # 0/1 mask on the 128-wide triangular sub-block only
                for hf in range(2):
                    exm = ex[:, hf * 512 + off:hf * 512 + off + 128]
                    nc.vector.tensor_mul(exm, exm,
                                         m01_t[:, hf * 128:(hf + 1) * 128])
            pending.append((kt, ex, off))
            if len(pending) > 2:  # lag 2: AV never waits on a fresh exp
                pop_one()
            for u in inject.get(kt, []):
                u()

        def flush_tail():
            while pending:
                pop_one()
            _stash(hp, qg, ti, avA, avB)
        return flush_tail

    def _stash(hp, qg, ti, avA, avB):
        hA, hB = 2 * hp, 2 * hp + 1
        if qg == LAST_QG and hp == HL // 2 - 1:
            # final slot: normalize inline via reciprocal + PE broadcast
            stg2 = rec_pool.tile([64, 512], F32, tag="stg2", name="stg2")
            nc.vector.memset(stg2, 1.0)
            for av, po, row in ((avA, 0, 0), (avB, 64, 32)):
                nc.vector.tensor_copy(
                    outT_t[ti][po:po + 64, qg * 512:(qg + 1) * 512],
                    av[0:64, :])
                nc.vector.tensor_copy(stg2[row:row + 1, :], av[64:65, :])
            rec2 = rec_pool.tile([64, 512], F32, tag="rec2", name="rec2")
            nc.vector.reciprocal_approx_fast(out=rec2, in_=stg2)
            recb2 = rec_pool.tile([64, 512], BF16, tag="recb2", name="recb2")
            nc.vector.tensor_copy(recb2, rec2)
            bc = av_pool.tile([128, 512], F32, tag="av", name="bc")
            nc.tensor.matmul(bc, lhsT=ones2, rhs=recb2, start=True, stop=True)
            for po in (0, 64):
                sl = outT_t[ti][po:po + 64, qg * 512:(qg + 1) * 512]
                nc.vector.tensor_mul(sl, sl, bc[po:po + 64, :])
        else:
            for av, h, po in ((avA, hA, 0), (avB, hB, 64)):
                nc.vector.tensor_copy(
                    outT_t[ti][po:po + 64, qg * 512:(qg + 1) * 512],
                    av[0:64, :])
                stg = rec_pool.tile([1, 512], F32, tag="stg", name="stg",
                                    bufs=4)
                nc.vector.tensor_copy(stg, av[64:65, :])
                nc.sync.dma_start(out=sums_dram[qg, h], in_=stg)

    # ---- batched normalization (DRAM-bounce broadcast) -------------------
    def _norm_heads(qg, heads):
        h0, nh = heads[0], len(heads)
        sums = rec_pool.tile([nh, 512], F32, tag=f"sums{nh}", name="sums")
        nc.sync.dma_start(out=sums, in_=sums_dram[qg, h0:h0 + nh])
        rec = rec_pool.tile([nh, 512], F32, tag=f"rec{nh}", name="rec")
        nc.vector.reciprocal_approx_fast(out=rec, in_=sums)
        recb = rec_pool.tile([nh, 512], BF16, tag=f"recb{nh}", name="recb")
        nc.vector.tensor_copy(recb, rec)
        nc.sync.dma_start(out=rec_dram[qg, h0:h0 + nh], in_=recb)
        for h in heads:
            ti, po = h // 2, 64 * (h % 2)
            bcs = rec_pool.tile([128, 512], BF16, tag="bcs", name="bcs")
            nc.sync.dma_start(
                out=bcs[po:po + 64, :],
                in_=rec_dram[qg, h:h + 1, :].to_broadcast([64, 512]))
            sl = outT_t[ti][po:po + 64, qg * 512:(qg + 1) * 512]
            nc.vector.tensor_mul(sl, sl, bcs[po:po + 64, :])

    def normalize(qg):
        _norm_heads(qg, list(range(HL)))

    def normalize_pair(qg, hp):
        _norm_heads(qg, [2 * hp, 2 * hp + 1])

    # ---- o-projection: y[s,:] partial ------------------------------------
    def oproj_h(st, hf):
        ps = fl_pool.tile([128, 512], F32, tag="fl", name="fps")
        for dt_ in range(NET):
            nc.tensor.matmul(
                ps,
                lhsT=outT_t[dt_][:, st * 128:(st + 1) * 128],
                rhs=woT_t[dt_][:, hf * 512:(hf + 1) * 512],
                start=(dt_ == 0), stop=(dt_ == NET - 1),
            )
        ysb = y_pool.tile([128, 512], BF16, tag="ysb", name="ysb", bufs=4)
        nc.vector.tensor_copy(ysb, ps)
        q = nc.sync if hf == 0 else nc.gpsimd
        q.dma_start(
            out=y[st * 128:(st + 1) * 128, hf * 512:(hf + 1) * 512],
            in_=ysb)

    def oproj(st):
        oproj_h(st, 0)
        oproj_h(st, 1)

    # ---- program order ----------------------------------------------------
    def qkQ(et, scg):
        return lambda: proj_qk(wqT_t, qT_t, et, scg)

    def qkK(et, scg):
        return lambda: proj_qk(wkT_t, kT_t, et, scg)

    def V(stp):
        return lambda: proj_v(stp)

    def O(st):
        return lambda: oproj(st)

    def N_(qg):
        return lambda: normalize(qg)

    proj_v(0)
    proj_v(1)
    proj_qk(wqT_t, qT_t, 0, 0)
    proj_qk(wkT_t, kT_t, 0, 0)

    # filler units injected inside each slot's kt loop (slot = (qg, hp) in
    # QG_ORDER-major, hp-minor order, s = visit index 0..15).  Deadlines:
    # qk(et,scg) before the first slot of a qg using scg that reads et;
    # v(stp) before any slot whose kt loop reaches st=2*stp; o(st) after
    # normalize of st's qg; normalize(qg) after all four qg stashes.
    fillers = {
        (0, 0): [qkQ(1, 0), qkK(1, 0)],
        (0, 1): [qkQ(2, 0), qkK(2, 0)],
        (0, 2): [qkQ(3, 0), qkK(3, 0)],
        (0, 3): [V(2), V(3)],
        (1, 0): [V(4), qkQ(0, 1), N_(0)],
        (1, 1): [V(5), qkK(0, 1)],
        (1, 2): [V(6), qkQ(1, 1)],
        (1, 3): [V(7), qkK(1, 1)],
        (3, 0): [qkQ(2, 1), qkK(2, 1), N_(1)],
        (3, 1): [qkQ(3, 1), qkK(3, 1)],
        (3, 2): [O(0), O(1), O(2)],
        (3, 3): [O(3), O(4), O(5)],
        (2, 0): [N_(3), O(6)],
        (2, 1): [O(7), O(12)],
        (2, 2): [O(13), O(14)],
        (2, 3): [O(15)],
    }
    post = {
        (2, 0): [lambda: normalize_pair(2, 0)],
        (2, 1): [lambda: normalize_pair(2, 1)],
        (2, 2): [lambda: normalize_pair(2, 2)],
    }
    for qg in QG_ORDER:
        for hp in range(HL // 2):
            flush_tail = attn(hp, qg, fillers.get((qg, hp), []))
            flush_tail()
            for f in post.get((qg, hp), []):
                f()
    for st in range(8, 12):  # qg2 (visited last) o-projections
        oproj(st)


def _build():
    nc = bacc.Bacc("TRN2", target_bir_lowering=False, debug=False,
                   num_devices=NCORES)
    xT = nc.dram_tensor("xT", [D, S], BF16, kind="ExternalInput").ap()
    wqT = nc.dram_tensor("wqT", [D, E], BF16, kind="ExternalInput").ap()
    wkT = nc.dram_tensor("wkT", [D, E], BF16, kind="ExternalInput").ap()
    wvT = nc.dram_tensor("wvT", [D, E], BF16, kind="ExternalInput").ap()
    woT = nc.dram_tensor("woT", [E, D], BF16, kind="ExternalInput").ap()
    m01 = nc.dram_tensor("m01", [128, 256], BF16, kind="ExternalInput").ap()
    y = nc.dram_tensor("y", [S, D], BF16, kind="ExternalOutput").ap()
    with tile.TileContext(nc) as tc:
        _mhsa_kernel(tc, y, xT, wqT, wkT, wvT, woT, m01)
    nc.compile()
    return nc


def get_compiled():
    global _compiled
    if _compiled is None:
        _compiled = _build()
    return _compiled


def _make_consts():
    # m01[k, qq] = 1 iff query qq >= key k within the 128-wide diagonal
    # sub-block; duplicated for the two packed heads.
    tri = np.triu(np.ones((128, 128), dtype=np.float32))
    m01 = np.concatenate([tri, tri], axis=1)
    return m01.astype(bf16)


def kernel(**inputs):
    global last_results
    x = np.asarray(inputs["in_features"], dtype=np.float32)
    w_q = np.asarray(inputs["w_q"], dtype=np.float32)
    w_k = np.asarray(inputs["w_k"], dtype=np.float32)
    w_v = np.asarray(inputs["w_v"], dtype=np.float32)
    w_o = np.asarray(inputs["w_o"], dtype=np.float32)

    nc = get_compiled()
    m01 = _make_consts()
    in_maps = []
    for c in range(NCORES):
        b, hg = divmod(c, 2)
        es = slice(hg * E, (hg + 1) * E)
        in_maps.append({
            "xT": x[b].T.astype(bf16),
            "wqT": w_q[es, :].T.astype(bf16),
            "wkT": w_k[es, :].T.astype(bf16),
            "wvT": w_v[es, :].T.astype(bf16),
            "woT": w_o[:, es].T.astype(bf16),
            "m01": m01,
        })
    res = run_bass_kernel_spmd(nc, in_maps, list(range(NCORES)))
    last_results = res
    y = np.zeros((B, S, D), dtype=np.float32)
    for c in range(NCORES):
        y[c // 2] += np.asarray(res.results[c]["y"], dtype=np.float32)
    return y


# revision 24
# speedup vs baseline: 5593.9295x; 5593.9295x over previous
"""Multi-head self-attention (causal) Trainium2 Bass/Tile kernel, 8-way SPMD.

Sharding: data-parallel over batch (4) x tensor-parallel over heads (2 groups
of 8 heads).  Core c handles batch c//2, head-group c%2.  Each core computes
q/k/v projections for its 512 local features, causal attention for its 8
heads, and a partial o-projection (contraction over its 512 features of the
attention output) giving a full-shape [S, D] partial that the host sums per
batch pair.

All matmul operands are bf16 (fp32 PSUM accumulation); softmax runs without
max-subtraction (scores ~ N(0,1) after the 1/8 scale, no overflow risk), with
exp on the scalar engine and the row-sum folded into the AV matmul via a ones
column appended to V.  Host pre-transposes inputs so no on-chip transposes
are needed:
  qT[e,s]  = wqT.T @ xT        (lhsT=wqT[d,e], rhs=xT[d,s])
  scoresT[sk,sq] = kT.T @ qT   (lhsT=kT[dk,sk], rhs=qT[dk,sq], K=64)
  avT[dk+1,sq]   = vaug.T @ expT  (lhsT=vaug[sk,65], rhs=expT[sk,sq])
  y[s,e]   = outT.T @ woT      (lhsT=outT[d,s], rhs=woT[d,e])

v3 vs v2 (307.5us):
 - masking back on DVE (PE tri-add cost more PE than it saved), but only on
   the trimmed 128-wide diagonal sub-block ([128,2,128] strided mul, ~194ns).
 - filler projection/oproj units are injected INSIDE each attention kt loop
   at spread points, so the PE never stalls on the exp stream's PSUM-buffer
   recycling (the v2 per-slot ~1.1-1.4us gaps).
 - query groups visit in order [0,1,3,2]: the ACT-heavy qg3 slots run while
   oproj filler still exists; qg2 (last) uses per-pair normalization and the
   final slot normalizes inline.
 - warmup back to 16 x N=512 matmuls (N=128 failed to trip the HAM window).
"""

from contextlib import ExitStack

import numpy as np
import ml_dtypes

import concourse.bass as bass
import concourse.tile as tile
from concourse import bacc, mybir
from concourse._compat import with_exitstack
from concourse.bass_utils import run_bass_kernel_spmd

B, S, D, H = 4, 2048, 1024, 16
DK = D // H          # 64
E = 512              # local features per core (8 heads)
HL = 8               # local heads
NCORES = 8
NDT = D // 128       # 8 d-tiles
NET = E // 128       # 4 e-tiles
NST = S // 128       # 16 s-tiles
NQG = S // 512       # 4 query groups

QG_ORDER = [0, 1, 3, 2]          # visit order; last visited gets inline norm
LAST_QG = QG_ORDER[-1]

F32 = mybir.dt.float32
BF16 = mybir.dt.bfloat16
bf16 = ml_dtypes.bfloat16

_compiled = None
last_results = None  # test harness introspection


@with_exitstack
def _mhsa_kernel(ctx: ExitStack, tc: tile.TileContext, y, xT, wqT, wkT, wvT,
                 woT, m01):
    nc = tc.nc

    consts = ctx.enter_context(tc.tile_pool(name="consts", bufs=1))
    ex_pool = ctx.enter_context(tc.tile_pool(name="ex", bufs=8))
    rec_pool = ctx.enter_context(tc.tile_pool(name="rec", bufs=2))
    y_pool = ctx.enter_context(tc.tile_pool(name="ysb", bufs=3))
    ps_pool = ctx.enter_context(tc.tile_pool(name="psmm", bufs=2, space="PSUM"))
    fl_pool = ctx.enter_context(tc.tile_pool(name="psfl", bufs=2, space="PSUM"))
    av_pool = ctx.enter_context(tc.tile_pool(name="psav", bufs=2, space="PSUM"))

    def ctile(shape, dt_, tg):
        return consts.tile(shape, dt_, tag=tg, name=tg)

    # ---- persistent SBUF tiles -------------------------------------------
    # x and weights live in single wide tiles (one batched strided DMA per
    # tensor; ~40 small descriptors serialized at ~600ns each on two queues
    # was stretching the input load to ~29us).  The per-d-tile names below
    # are views, so downstream indexing is unchanged.
    x_all = ctile([128, NDT * S], BF16, "xall")
    wq_all = ctile([128, NDT * E], BF16, "wqall")
    wk_all = ctile([128, NDT * E], BF16, "wkall")
    wv_all = ctile([128, NDT * E], BF16, "wvall")
    wo_all = ctile([128, NET * D], BF16, "woall")
    xT_t = [x_all[:, i * S:(i + 1) * S] for i in range(NDT)]
    wqT_t = [wq_all[:, i * E:(i + 1) * E] for i in range(NDT)]
    wkT_t = [wk_all[:, i * E:(i + 1) * E] for i in range(NDT)]
    wvT_t = [wv_all[:, i * E:(i + 1) * E] for i in range(NDT)]
    woT_t = [wo_all[:, i * D:(i + 1) * D] for i in range(NET)]
    qT_t = [ctile([128, S], BF16, f"qT{i}") for i in range(NET)]
    kT_t = [ctile([128, S], BF16, f"kT{i}") for i in range(NET)]
    vaug_t = [ctile([128, HL * (DK + 1)], BF16, f"vaug{i}") for i in range(NST)]
    outT_t = [ctile([128, S], BF16, f"outT{i}") for i in range(NET)]
    m01_t = ctile([128, 256], BF16, "m01")

    # ---- input loads, ordered so compute can start ASAP ------------------
    for c in range(4):
        nc.gpsimd.dma_start(
            out=x_all.rearrange("p (i s) -> p i s", s=S)[:, :,
                                                         c * 512:(c + 1) * 512],
            in_=xT.rearrange("(i p) s -> p i s", p=128)[:, :,
                                                        c * 512:(c + 1) * 512])
    nc.sync.dma_start(out=wv_all.rearrange("p (i e) -> p i e", e=E),
                      in_=wvT.rearrange("(i p) e -> p i e", p=128))
    nc.sync.dma_start(out=m01_t, in_=m01)
    nc.sync.dma_start(out=wq_all.rearrange("p (i e) -> p i e", e=E),
                      in_=wqT.rearrange("(i p) e -> p i e", p=128))
    nc.sync.dma_start(out=wk_all.rearrange("p (i e) -> p i e", e=E),
                      in_=wkT.rearrange("(i p) e -> p i e", p=128))
    nc.sync.dma_start(out=wo_all.rearrange("p (i e) -> p i e", e=D),
                      in_=woT.rearrange("(i p) e -> p i e", p=128))

    # ---- q/k projections: qT[e,s], kT[e,s] -------------------------------
    # Fillers use their own [128,512] PSUM pool so a filler matmul never
    # waits on a score tile's exp or another filler's DVE drain.
    def proj_qk_h(wt, dst, et, scg, hf):
        ps = fl_pool.tile([128, 512], F32, tag="fl", name="fps")
        s0 = scg * 1024 + hf * 512
        for dt_ in range(NDT):
            nc.tensor.matmul(
                ps,
                lhsT=wt[dt_][:, et * 128:(et + 1) * 128],
                rhs=xT_t[dt_][:, s0:s0 + 512],
                start=(dt_ == 0), stop=(dt_ == NDT - 1),
            )
        nc.vector.tensor_copy(dst[et][:, s0:s0 + 512], ps)

    def proj_qk(wt, dst, et, scg):
        proj_qk_h(wt, dst, et, scg, 0)
        proj_qk_h(wt, dst, et, scg, 1)

    # ---- v projection -> vaug tiles [128, 8*65] with ones columns --------
    def proj_v_st(st):
        ps = fl_pool.tile([128, 512], F32, tag="fl", name="fps")
        for dt_ in range(NDT):
            nc.tensor.matmul(
                ps,
                lhsT=xT_t[dt_][:, st * 128:(st + 1) * 128],
                rhs=wvT_t[dt_],
                start=(dt_ == 0), stop=(dt_ == NDT - 1),
            )
        nc.vector.memset(vaug_t[st], 1.0)
        nc.vector.tensor_copy(
            vaug_t[st].rearrange("p (h c) -> p h c", c=65)[:, :, 0:64],
            ps.rearrange("p (h c) -> p h c", c=64),
        )

    def proj_v(stp):
        proj_v_st(2 * stp)
        proj_v_st(2 * stp + 1)

    sums_dram = nc.dram_tensor("sums_bounce", [NQG, HL, 512], F32).ap()
    rec_dram = nc.dram_tensor("rec_bounce", [NQG, HL, 512], BF16).ap()

    # ones2: selector for the final pair's reciprocal broadcast matmul
    ones2 = ctile([64, 128], BF16, "ones2")
    nc.vector.memset(ones2, 0.0)
    nc.vector.memset(ones2[0:1, 0:64], 1.0)
    nc.vector.memset(ones2[32:33, 64:128], 1.0)


    # PE warm-up: ~6.8us of solid matmul streaming releases the HAM throttle
    warm = ctile([128, 512], BF16, "warm")
    nc.vector.memset(warm, 0.0)
    for _ in range(24):
        wps = ps_pool.tile([128, 512], F32, tag="mm", name="wps")
        nc.tensor.matmul(wps, lhsT=warm[:, 0:128], rhs=warm,
                         start=True, stop=True)

    # ---- attention for one (head-pair, query-group) ----------------------
    # AV runs as a col-tiled concurrent pair: head A -> av2[0:64] (array col
    # groups 0-1), head B -> av2[64:128] (groups 2-3); one ~225ns pass per
    # kt instead of two serial M=65 passes.  Softmax denominators become 4
    # concurrent M=1 col-tiled matmuls per kt PAIR (ones.T @ ex), landing at
    # den rows 0 (A,even kt), 32 (B,even), 64 (A,odd), 96 (B,odd); each
    # stream PSUM-accumulates across its kts.  The stash recombines
    # even+odd partials with one shifted copy + one add.
    def attn(hp, qg, units):
        ti = hp
        hA, hB = 2 * hp, 2 * hp + 1
        nk = 4 * qg + 4
        avA = av_pool.tile([65, 512], F32, tag="av", name="avA")
        avB = av_pool.tile([65, 512], F32, tag="av", name="avB")

        n_u = len(units)
        inject = {}
        for i in range(n_u):
            pt = (i + 1) * (nk - 1) // n_u if n_u else 0
            inject.setdefault(pt, []).append(units[i])

        def pop_one():
            kt, ex, off = pending.pop(0)
            for av, h in ((avA, hA), (avB, hB)):
                nc.tensor.matmul(
                    av[:, off:512],
                    lhsT=vaug_t[kt][:, h * 65:h * 65 + 65],
                    rhs=ex[:, (h & 1) * 512 + off:((h & 1) + 1) * 512],
                    start=(kt == 0), stop=(kt == nk - 1),
                    skip_group_check=True,
                )

        pending = []

        for kt in range(nk):
            j = kt - 4 * qg
            off = 128 * j if j >= 0 else 0
            diag = j >= 0
            ps = ps_pool.tile([128, 1024], F32, tag="mm", name="ps")
            for po in (0, 64):
                hf = po // 64
                nc.tensor.matmul(
                    ps[:, hf * 512 + off:(hf + 1) * 512],
                    lhsT=kT_t[ti][po:po + 64, kt * 128:(kt + 1) * 128],
                    rhs=qT_t[ti][po:po + 64, qg * 512 + off:(qg + 1) * 512],
                    start=True, stop=True,
                )
            ex = ex_pool.tile([128, 1024], BF16, tag="ex", name="ex")
            if off:
                ps_in = ps.rearrange("p (h q) -> p h q", q=512)[:, :, off:512]
                ex_out = ex.rearrange("p (h q) -> p h q", q=512)[:, :, off:512]
            else:
                ps_in, ex_out = ps, ex
            nc.scalar.activation(out=ex_out, in_=ps_in,
                                 func=mybir.ActivationFunctionType.Exp,
                                 scale=0.125)
            if diag:  # 0/1 mask on the 128-wide triangular sub-block only
                for hf in range(2):
                    exm = ex[:, hf * 512 + off:hf * 512 + off + 128]
                    nc.vector.tensor_mul(exm, exm,
                                         m01_t[:, hf * 128:(hf + 1) * 128])
            pending.append((kt, ex, off))
            if len(pending) > 2:  # lag 2: AV never waits on a fresh exp
                pop_one()
            for u in inject.get(kt, []):
                u()

        def flush_tail():
            while pending:
                pop_one()
            _stash(hp, qg, ti, avA, avB)
        return flush_tail

    def _stash(hp, qg, ti, avA, avB):
        hA, hB = 2 * hp, 2 * hp + 1
        if qg == LAST_QG and hp == HL // 2 - 1:
            # final slot: normalize inline via reciprocal + PE broadcast
            stg2 = rec_pool.tile([64, 512], F32, tag="stg2", name="stg2")
            nc.vector.memset(stg2, 1.0)
            for av, po, row in ((avA, 0, 0), (avB, 64, 32)):
                nc.vector.tensor_copy(
                    outT_t[ti][po:po + 64, qg * 512:(qg + 1) * 512],
                    av[0:64, :])
                nc.vector.tensor_copy(stg2[row:row + 1, :], av[64:65, :])
            rec2 = rec_pool.tile([64, 512], F32, tag="rec2", name="rec2")
            nc.vector.reciprocal_approx_fast(out=rec2, in_=stg2)
            recb2 = rec_pool.tile([64, 512], BF16, tag="recb2", name="recb2")
            nc.vector.tensor_copy(recb2, rec2)
            bc = av_pool.tile([128, 512], F32, tag="av", name="bc")
            nc.tensor.matmul(bc, lhsT=ones2, rhs=recb2, start=True, stop=True)
            for po in (0, 64):
                sl = outT_t[ti][po:po + 64, qg * 512:(qg + 1) * 512]
                nc.vector.tensor_mul(sl, sl, bc[po:po + 64, :])
        else:
            for av, h, po in ((avA, hA, 0), (avB, hB, 64)):
                nc.vector.tensor_copy(
                    outT_t[ti][po:po + 64, qg * 512:(qg + 1) * 512],
                    av[0:64, :])
                stg = rec_pool.tile([1, 512], F32, tag="stg", name="stg",
                                    bufs=4)
                nc.vector.tensor_copy(stg, av[64:65, :])
                nc.sync.dma_start(out=sums_dram[qg, h], in_=stg)

    # ---- batched normalization (DRAM-bounce broadcast) -------------------
    def _norm_heads(qg, heads):
        h0, nh = heads[0], len(heads)
        sums = rec_pool.tile([nh, 512], F32, tag=f"sums{nh}", name="sums")
        nc.sync.dma_start(out=sums, in_=sums_dram[qg, h0:h0 + nh])
        rec = rec_pool.tile([nh, 512], F32, tag=f"rec{nh}", name="rec")
        nc.vector.reciprocal_approx_fast(out=rec, in_=sums)
        recb = rec_pool.tile([nh, 512], BF16, tag=f"recb{nh}", name="recb")
        nc.vector.tensor_copy(recb, rec)
        nc.sync.dma_start(out=rec_dram[qg, h0:h0 + nh], in_=recb)
        for h in heads:
            ti, po = h // 2, 64 * (h % 2)
            bcs = rec_pool.tile([128, 512], BF16, tag="bcs", name="bcs")
            nc.sync.dma_start(
                out=bcs[po:po + 64, :],
                in_=rec_dram[qg, h:h + 1, :].to_broadcast([64, 512]))
            sl = outT_t[ti][po:po + 64, qg * 512:(qg + 1) * 512]
            nc.vector.tensor_mul(sl, sl, bcs[po:po + 64, :])

    def normalize(qg):
        _norm_heads(qg, list(range(HL)))

    def normalize_pair(qg, hp):
        _norm_heads(qg, [2 * hp, 2 * hp + 1])

    # ---- o-projection: y[s,:] partial ------------------------------------
    def oproj_h(st, hf):
        ps = fl_pool.tile([128, 512], F32, tag="fl", name="fps")
        for dt_ in range(NET):
            nc.tensor.matmul(
                ps,
                lhsT=outT_t[dt_][:, st * 128:(st + 1) * 128],
                rhs=woT_t[dt_][:, hf * 512:(hf + 1) * 512],
                start=(dt_ == 0), stop=(dt_ == NET - 1),
            )
        ysb = y_pool.tile([128, 512], BF16, tag="ysb", name="ysb", bufs=4)
        nc.vector.tensor_copy(ysb, ps)
        q = nc.sync if hf == 0 else nc.gpsimd
        q.dma_start(
            out=y[st * 128:(st + 1) * 128, hf * 512:(hf + 1) * 512],
            in_=ysb)

    def oproj(st):
        oproj_h(st, 0)
        oproj_h(st, 1)

    # ---- program order ----------------------------------------------------
    def qkQ(et, scg):
        return lambda: proj_qk(wqT_t, qT_t, et, scg)

    def qkK(et, scg):
        return lambda: proj_qk(wkT_t, kT_t, et, scg)

    def V(stp):
        return lambda: proj_v(stp)

    def O(st):
        return lambda: oproj(st)

    def N_(qg):
        return lambda: normalize(qg)

    proj_v(0)
    proj_v(1)
    proj_qk(wqT_t, qT_t, 0, 0)
    proj_qk(wkT_t, kT_t, 0, 0)

    # filler units injected inside each slot's kt loop (slot = (qg, hp) in
    # QG_ORDER-major, hp-minor order, s = visit index 0..15).  Deadlines:
    # qk(et,scg) before the first slot of a qg using scg that reads et;
    # v(stp) before any slot whose kt loop reaches st=2*stp; o(st) after
    # normalize of st's qg; normalize(qg) after all four qg stashes.
    fillers = {
        (0, 0): [qkQ(1, 0), qkK(1, 0)],
        (0, 1): [qkQ(2, 0), qkK(2, 0)],
        (0, 2): [qkQ(3, 0), qkK(3, 0)],
        (0, 3): [V(2), V(3)],
        (1, 0): [V(4), qkQ(0, 1), N_(0)],
        (1, 1): [V(5), qkK(0, 1)],
        (1, 2): [V(6), qkQ(1, 1)],
        (1, 3): [V(7), qkK(1, 1)],
        (3, 0): [qkQ(2, 1), qkK(2, 1), N_(1)],
        (3, 1): [qkQ(3, 1), qkK(3, 1)],
        (3, 2): [O(0), O(1), O(2)],
        (3, 3): [O(3), O(4), O(5)],
        (2, 0): [N_(3), O(6)],
        (2, 1): [O(7), O(12)],
        (2, 2): [O(13), O(14)],
        (2, 3): [O(15)],
    }
    post = {
        (2, 0): [lambda: normalize_pair(2, 0)],
        (2, 1): [lambda: normalize_pair(2, 1)],
        (2, 2): [lambda: normalize_pair(2, 2)],
    }
    for qg in QG_ORDER:
        for hp in range(HL // 2):
            flush_tail = attn(hp, qg, fillers.get((qg, hp), []))
            flush_tail()
            for f in post.get((qg, hp), []):
                f()
    for st in range(8, 12):  # qg2 (visited last) o-projections
        oproj(st)


def _build():
    nc = bacc.Bacc("TRN2", target_bir_lowering=False, debug=False,
                   num_devices=NCORES)
    xT = nc.dram_tensor("xT", [D, S], BF16, kind="ExternalInput").ap()
    wqT = nc.dram_tensor("wqT", [D, E], BF16, kind="ExternalInput").ap()
    wkT = nc.dram_tensor("wkT", [D, E], BF16, kind="ExternalInput").ap()
    wvT = nc.dram_tensor("wvT", [D, E], BF16, kind="ExternalInput").ap()
    woT = nc.dram_tensor("woT", [E, D], BF16, kind="ExternalInput").ap()
    m01 = nc.dram_tensor("m01", [128, 256], BF16, kind="ExternalInput").ap()
    y = nc.dram_tensor("y", [S, D], BF16, kind="ExternalOutput").ap()
    with tile.TileContext(nc) as tc:
        _mhsa_kernel(tc, y, xT, wqT, wkT, wvT, woT, m01)
    nc.compile()
    return nc


def get_compiled():
    global _compiled
    if _compiled is None:
        _compiled = _build()
    return _compiled


def _make_consts():
    # m01[k, qq] = 1 iff query qq >= key k within the 128-wide diagonal
    # sub-block; duplicated for the two packed heads.
    tri = np.triu(np.ones((128, 128), dtype=np.float32))
    m01 = np.concatenate([tri, tri], axis=1)
    return m01.astype(bf16)


def kernel(**inputs):
    global last_results
    x = np.asarray(inputs["in_features"], dtype=np.float32)
    w_q = np.asarray(inputs["w_q"], dtype=np.float32)
    w_k = np.asarray(inputs["w_k"], dtype=np.float32)
    w_v = np.asarray(inputs["w_v"], dtype=np.float32)
    w_o = np.asarray(inputs["w_o"], dtype=np.float32)

    nc = get_compiled()
    m01 = _make_consts()
    in_maps = []
    for c in range(NCORES):
        b, hg = divmod(c, 2)
        es = slice(hg * E, (hg + 1) * E)
        in_maps.append({
            "xT": x[b].T.astype(bf16),
            "wqT": w_q[es, :].T.astype(bf16),
            "wkT": w_k[es, :].T.astype(bf16),
            "wvT": w_v[es, :].T.astype(bf16),
            "woT": w_o[:, es].T.astype(bf16),
            "m01": m01,
        })
    res = run_bass_kernel_spmd(nc, in_maps, list(range(NCORES)))
    last_results = res
    y = np.zeros((B, S, D), dtype=np.float32)
    for c in range(NCORES):
        y[c // 2] += np.asarray(res.results[c]["y"], dtype=np.float32)
    return y


# revision 25
# speedup vs baseline: 6681.4465x; 1.1944x over previous
"""Multi-head self-attention (causal) Trainium2 Bass/Tile kernel, 8-way SPMD.

Sharding: data-parallel over batch (4) x tensor-parallel over heads (2 groups
of 8 heads).  Core c handles batch c//2, head-group c%2.  Each core computes
q/k/v projections for its 512 local features, causal attention for its 8
heads, and a partial o-projection (contraction over its 512 features of the
attention output) giving a full-shape [S, D] partial that the host sums per
batch pair.

All matmul operands are bf16 (fp32 PSUM accumulation); softmax runs without
max-subtraction (scores ~ N(0,1) after the 1/8 scale, no overflow risk), with
exp on the scalar engine and the row-sum folded into the AV matmul via a ones
column appended to V.  Host pre-transposes inputs so no on-chip transposes
are needed:
  qT[e,s]  = wqT.T @ xT        (lhsT=wqT[d,e], rhs=xT[d,s])
  scoresT[sk,sq] = kT.T @ qT   (lhsT=kT[dk,sk], rhs=qT[dk,sq], K=64)
  avT[dk+1,sq]   = vaug.T @ expT  (lhsT=vaug[sk,65], rhs=expT[sk,sq])
  y[s,e]   = outT.T @ woT      (lhsT=outT[d,s], rhs=woT[d,e])

v3 vs v2 (307.5us):
 - masking back on DVE (PE tri-add cost more PE than it saved), but only on
   the trimmed 128-wide diagonal sub-block ([128,2,128] strided mul, ~194ns).
 - filler projection/oproj units are injected INSIDE each attention kt loop
   at spread points, so the PE never stalls on the exp stream's PSUM-buffer
   recycling (the v2 per-slot ~1.1-1.4us gaps).
 - query groups visit in order [0,1,3,2]: the ACT-heavy qg3 slots run while
   oproj filler still exists; qg2 (last) uses per-pair normalization and the
   final slot normalizes inline.
 - warmup back to 16 x N=512 matmuls (N=128 failed to trip the HAM window).
"""

from contextlib import ExitStack

import numpy as np
import ml_dtypes

import concourse.bass as bass
import concourse.tile as tile
from concourse import bacc, mybir
from concourse._compat import with_exitstack
from concourse.bass_utils import run_bass_kernel_spmd

B, S, D, H = 4, 2048, 1024, 16
DK = D // H          # 64
E = 512              # local features per core (8 heads)
HL = 8               # local heads
NCORES = 8
NDT = D // 128       # 8 d-tiles
NET = E // 128       # 4 e-tiles
NST = S // 128       # 16 s-tiles
NQG = S // 512       # 4 query groups

QG_ORDER = [0, 1, 3, 2]          # visit order; last visited gets inline norm
LAST_QG = QG_ORDER[-1]

F32 = mybir.dt.float32
BF16 = mybir.dt.bfloat16
bf16 = ml_dtypes.bfloat16

_compiled = None
last_results = None  # test harness introspection


@with_exitstack
def _mhsa_kernel(ctx: ExitStack, tc: tile.TileContext, y, xT, wqT, wkT, wvT,
                 woT, m01):
    nc = tc.nc

    consts = ctx.enter_context(tc.tile_pool(name="consts", bufs=1))
    ex_pool = ctx.enter_context(tc.tile_pool(name="ex", bufs=8))
    rec_pool = ctx.enter_context(tc.tile_pool(name="rec", bufs=2))
    y_pool = ctx.enter_context(tc.tile_pool(name="ysb", bufs=3))
    ps_pool = ctx.enter_context(tc.tile_pool(name="psmm", bufs=2, space="PSUM"))
    fl_pool = ctx.enter_context(tc.tile_pool(name="psfl", bufs=2, space="PSUM"))
    av_pool = ctx.enter_context(tc.tile_pool(name="psav", bufs=2, space="PSUM"))

    def ctile(shape, dt_, tg):
        return consts.tile(shape, dt_, tag=tg, name=tg)

    # ---- persistent SBUF tiles -------------------------------------------
    # x and weights live in single wide tiles (one batched strided DMA per
    # tensor; ~40 small descriptors serialized at ~600ns each on two queues
    # was stretching the input load to ~29us).  The per-d-tile names below
    # are views, so downstream indexing is unchanged.
    x_all = ctile([128, NDT * S], BF16, "xall")
    wq_all = ctile([128, NDT * E], BF16, "wqall")
    wk_all = ctile([128, NDT * E], BF16, "wkall")
    wv_all = ctile([128, NDT * E], BF16, "wvall")
    wo_all = ctile([128, NET * D], BF16, "woall")
    xT_t = [x_all[:, i * S:(i + 1) * S] for i in range(NDT)]
    wqT_t = [wq_all[:, i * E:(i + 1) * E] for i in range(NDT)]
    wkT_t = [wk_all[:, i * E:(i + 1) * E] for i in range(NDT)]
    wvT_t = [wv_all[:, i * E:(i + 1) * E] for i in range(NDT)]
    woT_t = [wo_all[:, i * D:(i + 1) * D] for i in range(NET)]
    qT_t = [ctile([128, S], BF16, f"qT{i}") for i in range(NET)]
    kT_t = [ctile([128, S], BF16, f"kT{i}") for i in range(NET)]
    vaug_t = [ctile([128, HL * (DK + 1)], BF16, f"vaug{i}") for i in range(NST)]
    outT_t = [ctile([128, S], BF16, f"outT{i}") for i in range(NET)]
    m01_t = ctile([128, 256], BF16, "m01")

    # ---- input loads, ordered so compute can start ASAP ------------------
    for c in range(4):
        nc.gpsimd.dma_start(
            out=x_all.rearrange("p (i s) -> p i s", s=S)[:, :,
                                                         c * 512:(c + 1) * 512],
            in_=xT.rearrange("(i p) s -> p i s", p=128)[:, :,
                                                        c * 512:(c + 1) * 512])
    nc.sync.dma_start(out=wv_all.rearrange("p (i e) -> p i e", e=E),
                      in_=wvT.rearrange("(i p) e -> p i e", p=128))
    nc.sync.dma_start(out=m01_t, in_=m01)
    nc.sync.dma_start(out=wq_all.rearrange("p (i e) -> p i e", e=E),
                      in_=wqT.rearrange("(i p) e -> p i e", p=128))
    nc.sync.dma_start(out=wk_all.rearrange("p (i e) -> p i e", e=E),
                      in_=wkT.rearrange("(i p) e -> p i e", p=128))
    nc.sync.dma_start(out=wo_all.rearrange("p (i e) -> p i e", e=D),
                      in_=woT.rearrange("(i p) e -> p i e", p=128))

    # ---- q/k projections: qT[e,s], kT[e,s] -------------------------------
    # Fillers use their own [128,512] PSUM pool so a filler matmul never
    # waits on a score tile's exp or another filler's DVE drain.
    def proj_qk_h(wt, dst, et, scg, hf):
        ps = fl_pool.tile([128, 512], F32, tag="fl", name="fps")
        s0 = scg * 1024 + hf * 512
        for dt_ in range(NDT):
            nc.tensor.matmul(
                ps,
                lhsT=wt[dt_][:, et * 128:(et + 1) * 128],
                rhs=xT_t[dt_][:, s0:s0 + 512],
                start=(dt_ == 0), stop=(dt_ == NDT - 1),
            )
        nc.vector.tensor_copy(dst[et][:, s0:s0 + 512], ps)

    def proj_qk(wt, dst, et, scg):
        proj_qk_h(wt, dst, et, scg, 0)
        proj_qk_h(wt, dst, et, scg, 1)

    # ---- v projection -> vaug tiles [128, 8*65] with ones columns --------
    def proj_v_st(st):
        ps = fl_pool.tile([128, 512], F32, tag="fl", name="fps")
        for dt_ in range(NDT):
            nc.tensor.matmul(
                ps,
                lhsT=xT_t[dt_][:, st * 128:(st + 1) * 128],
                rhs=wvT_t[dt_],
                start=(dt_ == 0), stop=(dt_ == NDT - 1),
            )
        nc.vector.memset(vaug_t[st], 1.0)
        nc.vector.tensor_copy(
            vaug_t[st].rearrange("p (h c) -> p h c", c=65)[:, :, 0:64],
            ps.rearrange("p (h c) -> p h c", c=64),
        )

    def proj_v(stp):
        proj_v_st(2 * stp)
        proj_v_st(2 * stp + 1)

    sums_dram = nc.dram_tensor("sums_bounce", [NQG, HL, 512], F32).ap()
    rec_dram = nc.dram_tensor("rec_bounce", [NQG, HL, 512], BF16).ap()

    # ones2: selector for the final pair's reciprocal broadcast matmul
    ones2 = ctile([64, 128], BF16, "ones2")
    nc.vector.memset(ones2, 0.0)
    nc.vector.memset(ones2[0:1, 0:64], 1.0)
    nc.vector.memset(ones2[32:33, 64:128], 1.0)


    # PE warm-up: ~6.8us of solid matmul streaming releases the HAM throttle
    warm = ctile([128, 512], BF16, "warm")
    nc.vector.memset(warm, 0.0)
    for _ in range(16):
        wps = ps_pool.tile([128, 512], F32, tag="mm", name="wps")
        nc.tensor.matmul(wps, lhsT=warm[:, 0:128], rhs=warm,
                         start=True, stop=True)

    # ---- attention for one (head-pair, query-group) ----------------------
    # AV runs as a col-tiled concurrent pair: head A -> av2[0:64] (array col
    # groups 0-1), head B -> av2[64:128] (groups 2-3); one ~225ns pass per
    # kt instead of two serial M=65 passes.  Softmax denominators become 4
    # concurrent M=1 col-tiled matmuls per kt PAIR (ones.T @ ex), landing at
    # den rows 0 (A,even kt), 32 (B,even), 64 (A,odd), 96 (B,odd); each
    # stream PSUM-accumulates across its kts.  The stash recombines
    # even+odd partials with one shifted copy + one add.
    def attn(hp, qg, units):
        ti = hp
        hA, hB = 2 * hp, 2 * hp + 1
        nk = 4 * qg + 4
        avA = av_pool.tile([65, 512], F32, tag="av", name="avA")
        avB = av_pool.tile([65, 512], F32, tag="av", name="avB")

        n_u = len(units)
        inject = {}
        for i in range(n_u):
            pt = (i + 1) * (nk - 1) // n_u if n_u else 0
            inject.setdefault(pt, []).append(units[i])

        def pop_one():
            kt, ex, off = pending.pop(0)
            for av, h in ((avA, hA), (avB, hB)):
                nc.tensor.matmul(
                    av[:, off:512],
                    lhsT=vaug_t[kt][:, h * 65:h * 65 + 65],
                    rhs=ex[:, (h & 1) * 512 + off:((h & 1) + 1) * 512],
                    start=(kt == 0), stop=(kt == nk - 1),
                    skip_group_check=True,
                )

        pending = []

        for kt in range(nk):
            j = kt - 4 * qg
            off = 128 * j if j >= 0 else 0
            diag = j >= 0
            ps = ps_pool.tile([128, 1024], F32, tag="mm", name="ps")
            for po in (0, 64):
                hf = po // 64
                nc.tensor.matmul(
                    ps[:, hf * 512 + off:(hf + 1) * 512],
                    lhsT=kT_t[ti][po:po + 64, kt * 128:(kt + 1) * 128],
                    rhs=qT_t[ti][po:po + 64, qg * 512 + off:(qg + 1) * 512],
                    start=True, stop=True,
                )
            ex = ex_pool.tile([128, 1024], BF16, tag="ex", name="ex")
            if off:
                ps_in = ps.rearrange("p (h q) -> p h q", q=512)[:, :, off:512]
                ex_out = ex.rearrange("p (h q) -> p h q", q=512)[:, :, off:512]
            else:
                ps_in, ex_out = ps, ex
            nc.scalar.activation(out=ex_out, in_=ps_in,
                                 func=mybir.ActivationFunctionType.Exp,
                                 scale=0.125)
            if diag:  # 0/1 mask on the 128-wide triangular sub-block only
                for hf in range(2):
                    exm = ex[:, hf * 512 + off:hf * 512 + off + 128]
                    nc.vector.tensor_mul(exm, exm,
                                         m01_t[:, hf * 128:(hf + 1) * 128])
            pending.append((kt, ex, off))
            if len(pending) > 2:  # lag 2: AV never waits on a fresh exp
                pop_one()
            for u in inject.get(kt, []):
                u()

        def flush_tail():
            while pending:
                pop_one()
            _stash(hp, qg, ti, avA, avB)
        return flush_tail

    def _stash(hp, qg, ti, avA, avB):
        hA, hB = 2 * hp, 2 * hp + 1
        if qg == LAST_QG and hp == HL // 2 - 1:
            # final slot: normalize inline via reciprocal + PE broadcast
            stg2 = rec_pool.tile([64, 512], F32, tag="stg2", name="stg2")
            nc.vector.memset(stg2, 1.0)
            for av, po, row in ((avA, 0, 0), (avB, 64, 32)):
                nc.vector.tensor_copy(
                    outT_t[ti][po:po + 64, qg * 512:(qg + 1) * 512],
                    av[0:64, :])
                nc.vector.tensor_copy(stg2[row:row + 1, :], av[64:65, :])
            rec2 = rec_pool.tile([64, 512], F32, tag="rec2", name="rec2")
            nc.vector.reciprocal_approx_fast(out=rec2, in_=stg2)
            recb2 = rec_pool.tile([64, 512], BF16, tag="recb2", name="recb2")
            nc.vector.tensor_copy(recb2, rec2)
            bc = av_pool.tile([128, 512], F32, tag="av", name="bc")
            nc.tensor.matmul(bc, lhsT=ones2, rhs=recb2, start=True, stop=True)
            for po in (0, 64):
                sl = outT_t[ti][po:po + 64, qg * 512:(qg + 1) * 512]
                nc.vector.tensor_mul(sl, sl, bc[po:po + 64, :])
        else:
            for av, h, po in ((avA, hA, 0), (avB, hB, 64)):
                nc.vector.tensor_copy(
                    outT_t[ti][po:po + 64, qg * 512:(qg + 1) * 512],
                    av[0:64, :])
                stg = rec_pool.tile([1, 512], F32, tag="stg", name="stg",
                                    bufs=4)
                nc.vector.tensor_copy(stg, av[64:65, :])
                nc.sync.dma_start(out=sums_dram[qg, h], in_=stg)

    # ---- batched normalization (DRAM-bounce broadcast) -------------------
    def _norm_heads(qg, heads):
        h0, nh = heads[0], len(heads)
        sums = rec_pool.tile([nh, 512], F32, tag=f"sums{nh}", name="sums")
        nc.sync.dma_start(out=sums, in_=sums_dram[qg, h0:h0 + nh])
        rec = rec_pool.tile([nh, 512], F32, tag=f"rec{nh}", name="rec")
        nc.vector.reciprocal_approx_fast(out=rec, in_=sums)
        recb = rec_pool.tile([nh, 512], BF16, tag=f"recb{nh}", name="recb")
        nc.vector.tensor_copy(recb, rec)
        nc.sync.dma_start(out=rec_dram[qg, h0:h0 + nh], in_=recb)
        for h in heads:
            ti, po = h // 2, 64 * (h % 2)
            bcs = rec_pool.tile([128, 512], BF16, tag="bcs", name="bcs")
            nc.sync.dma_start(
                out=bcs[po:po + 64, :],
                in_=rec_dram[qg, h:h + 1, :].to_broadcast([64, 512]))
            sl = outT_t[ti][po:po + 64, qg * 512:(qg + 1) * 512]
            nc.vector.tensor_mul(sl, sl, bcs[po:po + 64, :])

    def normalize(qg):
        _norm_heads(qg, list(range(HL)))

    def normalize_pair(qg, hp):
        _norm_heads(qg, [2 * hp, 2 * hp + 1])

    # ---- o-projection: y[s,:] partial ------------------------------------
    def oproj_h(st, hf):
        ps = fl_pool.tile([128, 512], F32, tag="fl", name="fps")
        for dt_ in range(NET):
            nc.tensor.matmul(
                ps,
                lhsT=outT_t[dt_][:, st * 128:(st + 1) * 128],
                rhs=woT_t[dt_][:, hf * 512:(hf + 1) * 512],
                start=(dt_ == 0), stop=(dt_ == NET - 1),
            )
        ysb = y_pool.tile([128, 512], BF16, tag="ysb", name="ysb", bufs=4)
        nc.vector.tensor_copy(ysb, ps)
        q = nc.sync if hf == 0 else nc.gpsimd
        q.dma_start(
            out=y[st * 128:(st + 1) * 128, hf * 512:(hf + 1) * 512],
            in_=ysb)

    def oproj(st):
        oproj_h(st, 0)
        oproj_h(st, 1)

    # ---- program order ----------------------------------------------------
    def qkQ(et, scg):
        return lambda: proj_qk(wqT_t, qT_t, et, scg)

    def qkK(et, scg):
        return lambda: proj_qk(wkT_t, kT_t, et, scg)

    def V(stp):
        return lambda: proj_v(stp)

    def O(st):
        return lambda: oproj(st)

    def N_(qg):
        return lambda: normalize(qg)

    proj_v(0)
    proj_v(1)
    proj_qk(wqT_t, qT_t, 0, 0)
    proj_qk(wkT_t, kT_t, 0, 0)

    # filler units injected inside each slot's kt loop (slot = (qg, hp) in
    # QG_ORDER-major, hp-minor order, s = visit index 0..15).  Deadlines:
    # qk(et,scg) before the first slot of a qg using scg that reads et;
    # v(stp) before any slot whose kt loop reaches st=2*stp; o(st) after
    # normalize of st's qg; normalize(qg) after all four qg stashes.
    fillers = {
        (0, 0): [qkQ(1, 0), qkK(1, 0)],
        (0, 1): [qkQ(2, 0), qkK(2, 0)],
        (0, 2): [qkQ(3, 0), qkK(3, 0)],
        (0, 3): [V(2), V(3)],
        (1, 0): [V(4), qkQ(0, 1), N_(0)],
        (1, 1): [V(5), qkK(0, 1)],
        (1, 2): [V(6), qkQ(1, 1)],
        (1, 3): [V(7), qkK(1, 1)],
        (3, 0): [qkQ(2, 1), qkK(2, 1), N_(1)],
        (3, 1): [qkQ(3, 1), qkK(3, 1)],
        (3, 2): [O(0), O(1), O(2)],
        (3, 3): [O(3), O(4), O(5)],
        (2, 0): [N_(3), O(6)],
        (2, 1): [O(7), O(12)],
        (2, 2): [O(13), O(14)],
        (2, 3): [O(15)],
    }
    post = {
        (2, 0): [lambda: normalize_pair(2, 0)],
        (2, 1): [lambda: normalize_pair(2, 1)],
        (2, 2): [lambda: normalize_pair(2, 2)],
    }
    for qg in QG_ORDER:
        for hp in range(HL // 2):
            flush_tail = attn(hp, qg, fillers.get((qg, hp), []))
            flush_tail()
            for f in post.get((qg, hp), []):
                f()
    for st in range(8, 12):  # qg2 (visited last) o-projections
        oproj(st)


def _build():
    nc = bacc.Bacc("TRN2", target_bir_lowering=False, debug=False,
                   num_devices=NCORES)
    xT = nc.dram_tensor("xT", [D, S], BF16, kind="ExternalInput").ap()
    wqT = nc.dram_tensor("wqT", [D, E], BF16, kind="ExternalInput").ap()
    wkT = nc.dram_tensor("wkT", [D, E], BF16, kind="ExternalInput").ap()
    wvT = nc.dram_tensor("wvT", [D, E], BF16, kind="ExternalInput").ap()
    woT = nc.dram_tensor("woT", [E, D], BF16, kind="ExternalInput").ap()
    m01 = nc.dram_tensor("m01", [128, 256], BF16, kind="ExternalInput").ap()
    y = nc.dram_tensor("y", [S, D], BF16, kind="ExternalOutput").ap()
    with tile.TileContext(nc) as tc:
        _mhsa_kernel(tc, y, xT, wqT, wkT, wvT, woT, m01)
    nc.compile()
    return nc


def get_compiled():
    global _compiled
    if _compiled is None:
        _compiled = _build()
    return _compiled


def _make_consts():
    # m01[k, qq] = 1 iff query qq >= key k within the 128-wide diagonal
    # sub-block; duplicated for the two packed heads.
    tri = np.triu(np.ones((128, 128), dtype=np.float32))
    m01 = np.concatenate([tri, tri], axis=1)
    return m01.astype(bf16)


def kernel(**inputs):
    global last_results
    x = np.asarray(inputs["in_features"], dtype=np.float32)
    w_q = np.asarray(inputs["w_q"], dtype=np.float32)
    w_k = np.asarray(inputs["w_k"], dtype=np.float32)
    w_v = np.asarray(inputs["w_v"], dtype=np.float32)
    w_o = np.asarray(inputs["w_o"], dtype=np.float32)

    nc = get_compiled()
    m01 = _make_consts()
    in_maps = []
    for c in range(NCORES):
        b, hg = divmod(c, 2)
        es = slice(hg * E, (hg + 1) * E)
        in_maps.append({
            "xT": x[b].T.astype(bf16),
            "wqT": w_q[es, :].T.astype(bf16),
            "wkT": w_k[es, :].T.astype(bf16),
            "wvT": w_v[es, :].T.astype(bf16),
            "woT": w_o[:, es].T.astype(bf16),
            "m01": m01,
        })
    res = run_bass_kernel_spmd(nc, in_maps, list(range(NCORES)))
    last_results = res
    y = np.zeros((B, S, D), dtype=np.float32)
    for c in range(NCORES):
        y[c // 2] += np.asarray(res.results[c]["y"], dtype=np.float32)
    return y


# revision 32
# speedup vs baseline: 6849.5909x; 1.0252x over previous
"""Multi-head self-attention (causal) Trainium2 Bass/Tile kernel, 8-way SPMD.

Sharding: data-parallel over batch (4) x tensor-parallel over heads (2 groups
of 8 heads).  Core c handles batch c//2, head-group c%2.  Each core computes
q/k/v projections for its 512 local features, causal attention for its 8
heads, and a partial o-projection (contraction over its 512 features of the
attention output) giving a full-shape [S, D] partial that the host sums per
batch pair.

All matmul operands are bf16 (fp32 PSUM accumulation); softmax runs without
max-subtraction (scores ~ N(0,1) after the 1/8 scale, no overflow risk), with
exp on the scalar engine and the row-sum folded into the AV matmul via a ones
column appended to V.  Host pre-transposes inputs so no on-chip transposes
are needed:
  qT[e,s]  = wqT.T @ xT        (lhsT=wqT[d,e], rhs=xT[d,s])
  scoresT[sk,sq] = kT.T @ qT   (lhsT=kT[dk,sk], rhs=qT[dk,sq], K=64)
  avT[dk+1,sq]   = vaug.T @ expT  (lhsT=vaug[sk,65], rhs=expT[sk,sq])
  y[s,e]   = outT.T @ woT      (lhsT=outT[d,s], rhs=woT[d,e])

Optimizations vs the 316-322us baseline (now ~279us):
 - causal trim: diagonal-strip tiles only compute queries >= the tile's
   first key (score-MM N, exp AP, AV N, and the DVE mask all shrink; ~25%
   of attention work in the diag strips was masked-out waste).
 - masking on DVE over just the 128-wide triangular sub-block (two 2D
   [128,128] muls at 2x mode).
 - filler projection/oproj units are injected INSIDE each attention kt loop
   at end-biased spread points, with their own [128,512] PSUM pool, so the
   PE never stalls on the exp stream's score-buffer recycling and filler
   matmuls never wait on a previous filler's DVE drain.
 - query groups visit in order [0,1,3,2]: the ACT-heavy qg3 slots run while
   oproj filler still exists; qg2 (last) uses per-pair normalization and the
   final slot normalizes inline via a PE broadcast.
 - batched input DMAs: one strided-AP descriptor per weight tensor and four
   512-col slices for x (40 small descriptors at ~600ns issue each had
   stretched the input load to ~29us).
 - y output is bf16 (halves writeback; host accumulates partials in fp32).

Measurement note: the chip intermittently sits in a downclocked power state
(PE ~2.0GHz instead of 2.4) for whole runs; identical binaries measure
279us warm vs ~330us downclocked.  Compare versions only across repeated
runs.  (A col-tiled concurrent AV pair + 4-stream M=1 denominator variant
was tried and REVERTED: group-to-group LDW serialization makes the extra
denominator pass cost more than the concurrency saves; the ones-column AV
keeps denominators inside the same N-pass for free.)
"""

from contextlib import ExitStack

import numpy as np
import ml_dtypes

import concourse.bass as bass
import concourse.tile as tile
from concourse import bacc, mybir
from concourse._compat import with_exitstack
from concourse.bass_utils import run_bass_kernel_spmd

B, S, D, H = 4, 2048, 1024, 16
DK = D // H          # 64
E = 512              # local features per core (8 heads)
HL = 8               # local heads
NCORES = 8
NDT = D // 128       # 8 d-tiles
NET = E // 128       # 4 e-tiles
NST = S // 128       # 16 s-tiles
NQG = S // 512       # 4 query groups

QG_ORDER = [0, 1, 3, 2]          # visit order; last visited gets inline norm
LAST_QG = QG_ORDER[-1]

F32 = mybir.dt.float32
BF16 = mybir.dt.bfloat16
bf16 = ml_dtypes.bfloat16

_compiled = None
last_results = None  # test harness introspection


@with_exitstack
def _mhsa_kernel(ctx: ExitStack, tc: tile.TileContext, y, xT, wqT, wkT, wvT,
                 woT, m01):
    nc = tc.nc

    consts = ctx.enter_context(tc.tile_pool(name="consts", bufs=1))
    ex_pool = ctx.enter_context(tc.tile_pool(name="ex", bufs=8))
    rec_pool = ctx.enter_context(tc.tile_pool(name="rec", bufs=2))
    y_pool = ctx.enter_context(tc.tile_pool(name="ysb", bufs=3))
    ps_pool = ctx.enter_context(tc.tile_pool(name="psmm", bufs=2, space="PSUM"))
    fl_pool = ctx.enter_context(tc.tile_pool(name="psfl", bufs=2, space="PSUM"))
    av_pool = ctx.enter_context(tc.tile_pool(name="psav", bufs=2, space="PSUM"))

    def ctile(shape, dt_, tg):
        return consts.tile(shape, dt_, tag=tg, name=tg)

    # ---- persistent SBUF tiles -------------------------------------------
    # x and weights live in single wide tiles (one batched strided DMA per
    # tensor; ~40 small descriptors serialized at ~600ns each on two queues
    # was stretching the input load to ~29us).  The per-d-tile names below
    # are views, so downstream indexing is unchanged.
    x_all = ctile([128, NDT * S], BF16, "xall")
    wq_all = ctile([128, NDT * E], BF16, "wqall")
    wk_all = ctile([128, NDT * E], BF16, "wkall")
    wv_all = ctile([128, NDT * E], BF16, "wvall")
    wo_all = ctile([128, NET * D], BF16, "woall")
    xT_t = [x_all[:, i * S:(i + 1) * S] for i in range(NDT)]
    wqT_t = [wq_all[:, i * E:(i + 1) * E] for i in range(NDT)]
    wkT_t = [wk_all[:, i * E:(i + 1) * E] for i in range(NDT)]
    wvT_t = [wv_all[:, i * E:(i + 1) * E] for i in range(NDT)]
    woT_t = [wo_all[:, i * D:(i + 1) * D] for i in range(NET)]
    qT_t = [ctile([128, S], BF16, f"qT{i}") for i in range(NET)]
    kT_t = [ctile([128, S], BF16, f"kT{i}") for i in range(NET)]
    vaug_t = [ctile([128, HL * (DK + 1)], BF16, f"vaug{i}") for i in range(NST)]
    outT_t = [ctile([128, S], BF16, f"outT{i}") for i in range(NET)]
    m01_t = ctile([128, 256], BF16, "m01")

    # ---- input loads, ordered so compute can start ASAP ------------------
    for c in range(4):
        nc.gpsimd.dma_start(
            out=x_all.rearrange("p (i s) -> p i s", s=S)[:, :,
                                                         c * 512:(c + 1) * 512],
            in_=xT.rearrange("(i p) s -> p i s", p=128)[:, :,
                                                        c * 512:(c + 1) * 512])
    nc.sync.dma_start(out=wv_all.rearrange("p (i e) -> p i e", e=E),
                      in_=wvT.rearrange("(i p) e -> p i e", p=128))
    # wq/wk split by e-halves: qk(0,0)/qk(1,0) (the first consumers, via
    # et-column slices 0:256) unblock ~3us before the full tensors land
    for h in range(2):
        nc.sync.dma_start(
            out=wq_all.rearrange("p (i e) -> p i e", e=E)[:, :,
                                                          h * 256:(h + 1) * 256],
            in_=wqT.rearrange("(i p) e -> p i e", p=128)[:, :,
                                                         h * 256:(h + 1) * 256])
        nc.sync.dma_start(
            out=wk_all.rearrange("p (i e) -> p i e", e=E)[:, :,
                                                          h * 256:(h + 1) * 256],
            in_=wkT.rearrange("(i p) e -> p i e", p=128)[:, :,
                                                         h * 256:(h + 1) * 256])
    nc.sync.dma_start(out=m01_t, in_=m01)
    nc.sync.dma_start(out=wo_all.rearrange("p (i e) -> p i e", e=D),
                      in_=woT.rearrange("(i p) e -> p i e", p=128))

    # ---- q/k projections: qT[e,s], kT[e,s] -------------------------------
    # Fillers use their own [128,512] PSUM pool so a filler matmul never
    # waits on a score tile's exp or another filler's DVE drain.
    def proj_qk_h(wt, dst, et, scg, hf):
        ps = fl_pool.tile([128, 512], F32, tag="fl", name="fps")
        s0 = scg * 1024 + hf * 512
        for dt_ in range(NDT):
            nc.tensor.matmul(
                ps,
                lhsT=wt[dt_][:, et * 128:(et + 1) * 128],
                rhs=xT_t[dt_][:, s0:s0 + 512],
                start=(dt_ == 0), stop=(dt_ == NDT - 1),
            )
        nc.vector.tensor_copy(dst[et][:, s0:s0 + 512], ps)

    def proj_qk(wt, dst, et, scg):
        proj_qk_h(wt, dst, et, scg, 0)
        proj_qk_h(wt, dst, et, scg, 1)

    # ---- v projection -> vaug tiles [128, 8*65] with ones columns --------
    def proj_v_st(st):
        ps = fl_pool.tile([128, 512], F32, tag="fl", name="fps")
        for dt_ in range(NDT):
            nc.tensor.matmul(
                ps,
                lhsT=xT_t[dt_][:, st * 128:(st + 1) * 128],
                rhs=wvT_t[dt_],
                start=(dt_ == 0), stop=(dt_ == NDT - 1),
            )
        nc.vector.memset(vaug_t[st], 1.0)
        nc.vector.tensor_copy(
            vaug_t[st].rearrange("p (h c) -> p h c", c=65)[:, :, 0:64],
            ps.rearrange("p (h c) -> p h c", c=64),
        )

    def proj_v(stp):
        proj_v_st(2 * stp)
        proj_v_st(2 * stp + 1)

    sums_dram = nc.dram_tensor("sums_bounce", [NQG, HL, 512], F32).ap()
    rec_dram = nc.dram_tensor("rec_bounce", [NQG, HL, 512], BF16).ap()

    # ones2: selector for the final pair's reciprocal broadcast matmul
    ones2 = ctile([64, 128], BF16, "ones2")
    nc.vector.memset(ones2, 0.0)
    nc.vector.memset(ones2[0:1, 0:64], 1.0)
    nc.vector.memset(ones2[32:33, 64:128], 1.0)


    # PE warm-up: ~6.8us of solid matmul streaming releases the HAM throttle
    warm = ctile([128, 512], BF16, "warm")
    nc.vector.memset(warm, 0.0)
    for _ in range(22):
        wps = ps_pool.tile([128, 512], F32, tag="mm", name="wps")
        nc.tensor.matmul(wps, lhsT=warm[:, 0:128], rhs=warm,
                         start=True, stop=True)

    # ---- attention for one (head-pair, query-group) ----------------------
    # AV runs as a col-tiled concurrent pair: head A -> av2[0:64] (array col
    # groups 0-1), head B -> av2[64:128] (groups 2-3); one ~225ns pass per
    # kt instead of two serial M=65 passes.  Softmax denominators become 4
    # concurrent M=1 col-tiled matmuls per kt PAIR (ones.T @ ex), landing at
    # den rows 0 (A,even kt), 32 (B,even), 64 (A,odd), 96 (B,odd); each
    # stream PSUM-accumulates across its kts.  The stash recombines
    # even+odd partials with one shifted copy + one add.
    def attn(hp, qg, units):
        ti = hp
        hA, hB = 2 * hp, 2 * hp + 1
        nk = 4 * qg + 4
        avA = av_pool.tile([65, 512], F32, tag="av", name="avA")
        avB = av_pool.tile([65, 512], F32, tag="av", name="avB")

        n_u = len(units)
        inject = {}
        for i in range(n_u):
            pt = (i + 1) * (nk - 1) // n_u if n_u else 0
            inject.setdefault(pt, []).append(units[i])

        def pop_one():
            kt, ex, off = pending.pop(0)
            for av, h in ((avA, hA), (avB, hB)):
                nc.tensor.matmul(
                    av[:, off:512],
                    lhsT=vaug_t[kt][:, h * 65:h * 65 + 65],
                    rhs=ex[:, (h & 1) * 512 + off:((h & 1) + 1) * 512],
                    start=(kt == 0), stop=(kt == nk - 1),
                    skip_group_check=True,
                )

        pending = []

        for kt in range(nk):
            j = kt - 4 * qg
            off = 128 * j if j >= 0 else 0
            diag = j >= 0
            ps = ps_pool.tile([128, 1024], F32, tag="mm", name="ps")
            for po in (0, 64):
                hf = po // 64
                nc.tensor.matmul(
                    ps[:, hf * 512 + off:(hf + 1) * 512],
                    lhsT=kT_t[ti][po:po + 64, kt * 128:(kt + 1) * 128],
                    rhs=qT_t[ti][po:po + 64, qg * 512 + off:(qg + 1) * 512],
                    start=True, stop=True,
                )
            ex = ex_pool.tile([128, 1024], BF16, tag="ex", name="ex")
            if off:
                ps_in = ps.rearrange("p (h q) -> p h q", q=512)[:, :, off:512]
                ex_out = ex.rearrange("p (h q) -> p h q", q=512)[:, :, off:512]
            else:
                ps_in, ex_out = ps, ex
            nc.scalar.activation(out=ex_out, in_=ps_in,
                                 func=mybir.ActivationFunctionType.Exp,
                                 scale=0.125)
            if diag:  # 0/1 mask on the 128-wide triangular sub-block only
                for hf in range(2):
                    exm = ex[:, hf * 512 + off:hf * 512 + off + 128]
                    nc.vector.tensor_mul(exm, exm,
                                         m01_t[:, hf * 128:(hf + 1) * 128])
            pending.append((kt, ex, off))
            if len(pending) > 2:  # lag 2: AV never waits on a fresh exp
                pop_one()
            for u in inject.get(kt, []):
                u()

        def flush_tail():
            while pending:
                pop_one()
            _stash(hp, qg, ti, avA, avB)
        return flush_tail

    def _stash(hp, qg, ti, avA, avB):
        hA, hB = 2 * hp, 2 * hp + 1
        if qg == LAST_QG and hp == HL // 2 - 1:
            # final slot: normalize inline via reciprocal + PE broadcast
            stg2 = rec_pool.tile([64, 512], F32, tag="stg2", name="stg2")
            nc.vector.memset(stg2, 1.0)
            for av, po, row in ((avA, 0, 0), (avB, 64, 32)):
                nc.vector.tensor_copy(
                    outT_t[ti][po:po + 64, qg * 512:(qg + 1) * 512],
                    av[0:64, :])
                nc.vector.tensor_copy(stg2[row:row + 1, :], av[64:65, :])
            rec2 = rec_pool.tile([64, 512], F32, tag="rec2", name="rec2")
            nc.vector.reciprocal_approx_fast(out=rec2, in_=stg2)
            recb2 = rec_pool.tile([64, 512], BF16, tag="recb2", name="recb2")
            nc.vector.tensor_copy(recb2, rec2)
            bc = av_pool.tile([128, 512], F32, tag="av", name="bc")
            nc.tensor.matmul(bc, lhsT=ones2, rhs=recb2, start=True, stop=True)
            for po in (0, 64):
                sl = outT_t[ti][po:po + 64, qg * 512:(qg + 1) * 512]
                nc.vector.tensor_mul(sl, sl, bc[po:po + 64, :])
        else:
            for av, h, po in ((avA, hA, 0), (avB, hB, 64)):
                nc.vector.tensor_copy(
                    outT_t[ti][po:po + 64, qg * 512:(qg + 1) * 512],
                    av[0:64, :])
                stg = rec_pool.tile([1, 512], F32, tag="stg", name="stg",
                                    bufs=4)
                nc.vector.tensor_copy(stg, av[64:65, :])
                nc.sync.dma_start(out=sums_dram[qg, h], in_=stg)

    # ---- batched normalization (DRAM-bounce broadcast) -------------------
    def _norm_heads(qg, heads):
        h0, nh = heads[0], len(heads)
        sums = rec_pool.tile([nh, 512], F32, tag=f"sums{nh}", name="sums")
        nc.sync.dma_start(out=sums, in_=sums_dram[qg, h0:h0 + nh])
        rec = rec_pool.tile([nh, 512], F32, tag=f"rec{nh}", name="rec")
        nc.vector.reciprocal_approx_fast(out=rec, in_=sums)
        recb = rec_pool.tile([nh, 512], BF16, tag=f"recb{nh}", name="recb")
        nc.vector.tensor_copy(recb, rec)
        nc.sync.dma_start(out=rec_dram[qg, h0:h0 + nh], in_=recb)
        for h in heads:
            ti, po = h // 2, 64 * (h % 2)
            bcs = rec_pool.tile([128, 512], BF16, tag="bcs", name="bcs")
            nc.sync.dma_start(
                out=bcs[po:po + 64, :],
                in_=rec_dram[qg, h:h + 1, :].to_broadcast([64, 512]))
            sl = outT_t[ti][po:po + 64, qg * 512:(qg + 1) * 512]
            nc.vector.tensor_mul(sl, sl, bcs[po:po + 64, :])

    def normalize(qg):
        _norm_heads(qg, list(range(HL)))

    def normalize_pair(qg, hp):
        _norm_heads(qg, [2 * hp, 2 * hp + 1])

    # ---- o-projection: y[s,:] partial ------------------------------------
    def oproj_h(st, hf):
        ps = fl_pool.tile([128, 512], F32, tag="fl", name="fps")
        for dt_ in range(NET):
            nc.tensor.matmul(
                ps,
                lhsT=outT_t[dt_][:, st * 128:(st + 1) * 128],
                rhs=woT_t[dt_][:, hf * 512:(hf + 1) * 512],
                start=(dt_ == 0), stop=(dt_ == NET - 1),
            )
        ysb = y_pool.tile([128, 512], BF16, tag="ysb", name="ysb", bufs=4)
        nc.vector.tensor_copy(ysb, ps)
        q = nc.sync if hf == 0 else nc.gpsimd
        q.dma_start(
            out=y[st * 128:(st + 1) * 128, hf * 512:(hf + 1) * 512],
            in_=ysb)

    def oproj(st):
        oproj_h(st, 0)
        oproj_h(st, 1)

    # endgame helpers: o-projection split so the et0..2 partial sums run
    # BEFORE the final slot's inline normalization (only et3 depends on it),
    # hiding the serial reciprocal/broadcast chain behind PE work
    def oproj_part(st, hf, pool):
        ps = pool.tile([128, 512], F32,
                       tag="fl" if pool is fl_pool else "mm", name="fps")
        for dt_ in range(3):
            nc.tensor.matmul(
                ps,
                lhsT=outT_t[dt_][:, st * 128:(st + 1) * 128],
                rhs=woT_t[dt_][:, hf * 512:(hf + 1) * 512],
                start=(dt_ == 0), stop=False, skip_group_check=True,
            )
        return ps

    def oproj_fin(st, hf, ps):
        nc.tensor.matmul(
            ps,
            lhsT=outT_t[3][:, st * 128:(st + 1) * 128],
            rhs=woT_t[3][:, hf * 512:(hf + 1) * 512],
            start=False, stop=True, skip_group_check=True,
        )
        ysb = y_pool.tile([128, 512], BF16, tag="ysb", name="ysb", bufs=4)
        nc.vector.tensor_copy(ysb, ps)
        q = nc.sync if hf == 0 else nc.gpsimd
        q.dma_start(
            out=y[st * 128:(st + 1) * 128, hf * 512:(hf + 1) * 512],
            in_=ysb)

    # ---- program order ----------------------------------------------------
    def qkQ(et, scg):
        return lambda: proj_qk(wqT_t, qT_t, et, scg)

    def qkK(et, scg):
        return lambda: proj_qk(wkT_t, kT_t, et, scg)

    def V(stp):
        return lambda: proj_v(stp)

    def O(st):
        return lambda: oproj(st)

    def N_(qg):
        return lambda: normalize(qg)

    held = {}

    def OP(st, hf):
        def f():
            held[(st, hf)] = oproj_part(st, hf, fl_pool)
        return f

    proj_v(0)
    proj_v(1)
    proj_qk(wqT_t, qT_t, 0, 0)
    proj_qk(wkT_t, kT_t, 0, 0)

    # filler units injected inside each slot's kt loop (slot = (qg, hp) in
    # QG_ORDER-major, hp-minor order, s = visit index 0..15).  Deadlines:
    # qk(et,scg) before the first slot of a qg using scg that reads et;
    # v(stp) before any slot whose kt loop reaches st=2*stp; o(st) after
    # normalize of st's qg; normalize(qg) after all four qg stashes.
    fillers = {
        (0, 0): [qkQ(1, 0), qkK(1, 0)],
        (0, 1): [qkQ(2, 0), qkK(2, 0)],
        (0, 2): [qkQ(3, 0), qkK(3, 0)],
        (0, 3): [V(2), V(3)],
        (1, 0): [V(4), qkQ(0, 1), N_(0)],
        (1, 1): [V(5), qkK(0, 1)],
        (1, 2): [V(6), qkQ(1, 1)],
        (1, 3): [V(7), qkK(1, 1)],
        (3, 0): [qkQ(2, 1), qkK(2, 1), N_(1)],
        (3, 1): [qkQ(3, 1), qkK(3, 1)],
        (3, 2): [O(0), O(1), O(2)],
        (3, 3): [O(3), O(4), O(5)],
        (2, 0): [N_(3), O(6)],
        (2, 1): [O(7), O(12)],
        (2, 2): [O(13), O(14)],
        (2, 3): [O(15), OP(8, 0), OP(8, 1)],
    }
    post = {
        (2, 0): [lambda: normalize_pair(2, 0)],
        (2, 1): [lambda: normalize_pair(2, 1)],
        (2, 2): [lambda: normalize_pair(2, 2)],
    }
    for qg in QG_ORDER:
        for hp in range(HL // 2):
            flush_tail = attn(hp, qg, fillers.get((qg, hp), []))
            flush_tail()
            for f in post.get((qg, hp), []):
                f()
    # endgame: st8's et0..2 partials were prefetched in the (2,3) slot; run
    # st9's partials now (they only need np(2,0..2)) so the PE works through
    # the final slot's normalization chain, then finish et3 + drains.
    held[(9, 0)] = oproj_part(9, 0, ps_pool)
    held[(9, 1)] = oproj_part(9, 1, ps_pool)
    for st in (8, 9):
        for hf in (0, 1):
            oproj_fin(st, hf, held.pop((st, hf)))
    oproj(10)
    oproj(11)


def _build():
    nc = bacc.Bacc("TRN2", target_bir_lowering=False, debug=False,
                   num_devices=NCORES)
    xT = nc.dram_tensor("xT", [D, S], BF16, kind="ExternalInput").ap()
    wqT = nc.dram_tensor("wqT", [D, E], BF16, kind="ExternalInput").ap()
    wkT = nc.dram_tensor("wkT", [D, E], BF16, kind="ExternalInput").ap()
    wvT = nc.dram_tensor("wvT", [D, E], BF16, kind="ExternalInput").ap()
    woT = nc.dram_tensor("woT", [E, D], BF16, kind="ExternalInput").ap()
    m01 = nc.dram_tensor("m01", [128, 256], BF16, kind="ExternalInput").ap()
    y = nc.dram_tensor("y", [S, D], BF16, kind="ExternalOutput").ap()
    with tile.TileContext(nc) as tc:
        _mhsa_kernel(tc, y, xT, wqT, wkT, wvT, woT, m01)
    nc.compile()
    return nc


def get_compiled():
    global _compiled
    if _compiled is None:
        _compiled = _build()
    return _compiled


def _make_consts():
    # m01[k, qq] = 1 iff query qq >= key k within the 128-wide diagonal
    # sub-block; duplicated for the two packed heads.
    tri = np.triu(np.ones((128, 128), dtype=np.float32))
    m01 = np.concatenate([tri, tri], axis=1)
    return m01.astype(bf16)


def kernel(**inputs):
    global last_results
    x = np.asarray(inputs["in_features"], dtype=np.float32)
    w_q = np.asarray(inputs["w_q"], dtype=np.float32)
    w_k = np.asarray(inputs["w_k"], dtype=np.float32)
    w_v = np.asarray(inputs["w_v"], dtype=np.float32)
    w_o = np.asarray(inputs["w_o"], dtype=np.float32)

    nc = get_compiled()
    m01 = _make_consts()
    in_maps = []
    for c in range(NCORES):
        b, hg = divmod(c, 2)
        es = slice(hg * E, (hg + 1) * E)
        in_maps.append({
            "xT": x[b].T.astype(bf16),
            "wqT": w_q[es, :].T.astype(bf16),
            "wkT": w_k[es, :].T.astype(bf16),
            "wvT": w_v[es, :].T.astype(bf16),
            "woT": w_o[:, es].T.astype(bf16),
            "m01": m01,
        })
    res = run_bass_kernel_spmd(nc, in_maps, list(range(NCORES)))
    last_results = res
    y = np.zeros((B, S, D), dtype=np.float32)
    for c in range(NCORES):
        y[c // 2] += np.asarray(res.results[c]["y"], dtype=np.float32)
    return y


# revision 33
# speedup vs baseline: 6870.1108x; 1.0030x over previous
"""Multi-head self-attention (causal) Trainium2 Bass/Tile kernel, 8-way SPMD.

Sharding: data-parallel over batch (4) x tensor-parallel over heads (2 groups
of 8 heads).  Core c handles batch c//2, head-group c%2.  Each core computes
q/k/v projections for its 512 local features, causal attention for its 8
heads, and a partial o-projection (contraction over its 512 features of the
attention output) giving a full-shape [S, D] partial that the host sums per
batch pair.

All matmul operands are bf16 (fp32 PSUM accumulation); softmax runs without
max-subtraction (scores ~ N(0,1) after the 1/8 scale, no overflow risk), with
exp on the scalar engine and the row-sum folded into the AV matmul via a ones
column appended to V.  Host pre-transposes inputs so no on-chip transposes
are needed:
  qT[e,s]  = wqT.T @ xT        (lhsT=wqT[d,e], rhs=xT[d,s])
  scoresT[sk,sq] = kT.T @ qT   (lhsT=kT[dk,sk], rhs=qT[dk,sq], K=64)
  avT[dk+1,sq]   = vaug.T @ expT  (lhsT=vaug[sk,65], rhs=expT[sk,sq])
  y[s,e]   = outT.T @ woT      (lhsT=outT[d,s], rhs=woT[d,e])

Optimizations vs the 316-322us baseline (now ~279us):
 - causal trim: diagonal-strip tiles only compute queries >= the tile's
   first key (score-MM N, exp AP, AV N, and the DVE mask all shrink; ~25%
   of attention work in the diag strips was masked-out waste).
 - masking on DVE over just the 128-wide triangular sub-block (two 2D
   [128,128] muls at 2x mode).
 - filler projection/oproj units are injected INSIDE each attention kt loop
   at end-biased spread points, with their own [128,512] PSUM pool, so the
   PE never stalls on the exp stream's score-buffer recycling and filler
   matmuls never wait on a previous filler's DVE drain.
 - query groups visit in order [0,1,3,2]: the ACT-heavy qg3 slots run while
   oproj filler still exists; qg2 (last) uses per-pair normalization and the
   final slot normalizes inline via a PE broadcast.
 - batched input DMAs: one strided-AP descriptor per weight tensor and four
   512-col slices for x (40 small descriptors at ~600ns issue each had
   stretched the input load to ~29us).
 - y output is bf16 (halves writeback; host accumulates partials in fp32).

Measurement note: the chip intermittently sits in a downclocked power state
(PE ~2.0GHz instead of 2.4) for whole runs; identical binaries measure
279us warm vs ~330us downclocked.  Compare versions only across repeated
runs.  (A col-tiled concurrent AV pair + 4-stream M=1 denominator variant
was tried and REVERTED: group-to-group LDW serialization makes the extra
denominator pass cost more than the concurrency saves; the ones-column AV
keeps denominators inside the same N-pass for free.)
"""

from contextlib import ExitStack

import numpy as np
import ml_dtypes

import concourse.bass as bass
import concourse.tile as tile
from concourse import bacc, mybir
from concourse._compat import with_exitstack
from concourse.bass_utils import run_bass_kernel_spmd

B, S, D, H = 4, 2048, 1024, 16
DK = D // H          # 64
E = 512              # local features per core (8 heads)
HL = 8               # local heads
NCORES = 8
NDT = D // 128       # 8 d-tiles
NET = E // 128       # 4 e-tiles
NST = S // 128       # 16 s-tiles
NQG = S // 512       # 4 query groups

QG_ORDER = [0, 1, 3, 2]          # visit order; last visited gets inline norm
LAST_QG = QG_ORDER[-1]

F32 = mybir.dt.float32
BF16 = mybir.dt.bfloat16
bf16 = ml_dtypes.bfloat16

_compiled = None
last_results = None  # test harness introspection


@with_exitstack
def _mhsa_kernel(ctx: ExitStack, tc: tile.TileContext, y, xT, wqT, wkT, wvT,
                 woT, m01):
    nc = tc.nc

    consts = ctx.enter_context(tc.tile_pool(name="consts", bufs=1))
    ex_pool = ctx.enter_context(tc.tile_pool(name="ex", bufs=8))
    rec_pool = ctx.enter_context(tc.tile_pool(name="rec", bufs=2))
    y_pool = ctx.enter_context(tc.tile_pool(name="ysb", bufs=3))
    ps_pool = ctx.enter_context(tc.tile_pool(name="psmm", bufs=2, space="PSUM"))
    fl_pool = ctx.enter_context(tc.tile_pool(name="psfl", bufs=2, space="PSUM"))
    av_pool = ctx.enter_context(tc.tile_pool(name="psav", bufs=2, space="PSUM"))

    def ctile(shape, dt_, tg):
        return consts.tile(shape, dt_, tag=tg, name=tg)

    # ---- persistent SBUF tiles -------------------------------------------
    # x and weights live in single wide tiles (one batched strided DMA per
    # tensor; ~40 small descriptors serialized at ~600ns each on two queues
    # was stretching the input load to ~29us).  The per-d-tile names below
    # are views, so downstream indexing is unchanged.
    x_all = ctile([128, NDT * S], BF16, "xall")
    wq_all = ctile([128, NDT * E], BF16, "wqall")
    wk_all = ctile([128, NDT * E], BF16, "wkall")
    wv_all = ctile([128, NDT * E], BF16, "wvall")
    wo_all = ctile([128, NET * D], BF16, "woall")
    xT_t = [x_all[:, i * S:(i + 1) * S] for i in range(NDT)]
    wqT_t = [wq_all[:, i * E:(i + 1) * E] for i in range(NDT)]
    wkT_t = [wk_all[:, i * E:(i + 1) * E] for i in range(NDT)]
    wvT_t = [wv_all[:, i * E:(i + 1) * E] for i in range(NDT)]
    woT_t = [wo_all[:, i * D:(i + 1) * D] for i in range(NET)]
    qT_t = [ctile([128, S], BF16, f"qT{i}") for i in range(NET)]
    kT_t = [ctile([128, S], BF16, f"kT{i}") for i in range(NET)]
    vaug_t = [ctile([128, HL * (DK + 1)], BF16, f"vaug{i}") for i in range(NST)]
    outT_t = [ctile([128, S], BF16, f"outT{i}") for i in range(NET)]
    m01_t = ctile([128, 256], BF16, "m01")

    # ---- input loads, ordered so compute can start ASAP ------------------
    for c in range(4):
        nc.gpsimd.dma_start(
            out=x_all.rearrange("p (i s) -> p i s", s=S)[:, :,
                                                         c * 512:(c + 1) * 512],
            in_=xT.rearrange("(i p) s -> p i s", p=128)[:, :,
                                                        c * 512:(c + 1) * 512])
    nc.sync.dma_start(out=wv_all.rearrange("p (i e) -> p i e", e=E),
                      in_=wvT.rearrange("(i p) e -> p i e", p=128))
    # wq/wk split by e-halves: qk(0,0)/qk(1,0) (the first consumers, via
    # et-column slices 0:256) unblock ~3us before the full tensors land
    for h in range(2):
        nc.sync.dma_start(
            out=wq_all.rearrange("p (i e) -> p i e", e=E)[:, :,
                                                          h * 256:(h + 1) * 256],
            in_=wqT.rearrange("(i p) e -> p i e", p=128)[:, :,
                                                         h * 256:(h + 1) * 256])
        nc.sync.dma_start(
            out=wk_all.rearrange("p (i e) -> p i e", e=E)[:, :,
                                                          h * 256:(h + 1) * 256],
            in_=wkT.rearrange("(i p) e -> p i e", p=128)[:, :,
                                                         h * 256:(h + 1) * 256])
    nc.sync.dma_start(out=m01_t, in_=m01)
    nc.sync.dma_start(out=wo_all.rearrange("p (i e) -> p i e", e=D),
                      in_=woT.rearrange("(i p) e -> p i e", p=128))

    # ---- q/k projections: qT[e,s], kT[e,s] -------------------------------
    # Fillers use their own [128,512] PSUM pool so a filler matmul never
    # waits on a score tile's exp or another filler's DVE drain.
    def proj_qk_h(wt, dst, et, scg, hf):
        ps = fl_pool.tile([128, 512], F32, tag="fl", name="fps")
        s0 = scg * 1024 + hf * 512
        for dt_ in range(NDT):
            nc.tensor.matmul(
                ps,
                lhsT=wt[dt_][:, et * 128:(et + 1) * 128],
                rhs=xT_t[dt_][:, s0:s0 + 512],
                start=(dt_ == 0), stop=(dt_ == NDT - 1),
            )
        nc.vector.tensor_copy(dst[et][:, s0:s0 + 512], ps)

    def proj_qk(wt, dst, et, scg):
        proj_qk_h(wt, dst, et, scg, 0)
        proj_qk_h(wt, dst, et, scg, 1)

    # ---- v projection -> vaug tiles [128, 8*65] with ones columns --------
    def proj_v_st(st):
        ps = fl_pool.tile([128, 512], F32, tag="fl", name="fps")
        for dt_ in range(NDT):
            nc.tensor.matmul(
                ps,
                lhsT=xT_t[dt_][:, st * 128:(st + 1) * 128],
                rhs=wvT_t[dt_],
                start=(dt_ == 0), stop=(dt_ == NDT - 1),
            )
        nc.vector.memset(vaug_t[st], 1.0)
        nc.vector.tensor_copy(
            vaug_t[st].rearrange("p (h c) -> p h c", c=65)[:, :, 0:64],
            ps.rearrange("p (h c) -> p h c", c=64),
        )

    def proj_v(stp):
        proj_v_st(2 * stp)
        proj_v_st(2 * stp + 1)

    sums_dram = nc.dram_tensor("sums_bounce", [NQG, HL, 512], F32).ap()
    rec_dram = nc.dram_tensor("rec_bounce", [NQG, HL, 512], BF16).ap()

    # ones2: selector for the final pair's reciprocal broadcast matmul
    ones2 = ctile([64, 128], BF16, "ones2")
    nc.vector.memset(ones2, 0.0)
    nc.vector.memset(ones2[0:1, 0:64], 1.0)
    nc.vector.memset(ones2[32:33, 64:128], 1.0)


    # PE warm-up: ~6.8us of solid matmul streaming releases the HAM throttle
    warm = ctile([128, 512], BF16, "warm")
    nc.vector.memset(warm, 0.0)
    for _ in range(22):
        wps = ps_pool.tile([128, 512], F32, tag="mm", name="wps")
        nc.tensor.matmul(wps, lhsT=warm[:, 0:128], rhs=warm,
                         start=True, stop=True)

    # ---- attention for one (head-pair, query-group) ----------------------
    # AV runs as a col-tiled concurrent pair: head A -> av2[0:64] (array col
    # groups 0-1), head B -> av2[64:128] (groups 2-3); one ~225ns pass per
    # kt instead of two serial M=65 passes.  Softmax denominators become 4
    # concurrent M=1 col-tiled matmuls per kt PAIR (ones.T @ ex), landing at
    # den rows 0 (A,even kt), 32 (B,even), 64 (A,odd), 96 (B,odd); each
    # stream PSUM-accumulates across its kts.  The stash recombines
    # even+odd partials with one shifted copy + one add.
    def attn(hp, qg, units):
        ti = hp
        hA, hB = 2 * hp, 2 * hp + 1
        nk = 4 * qg + 4
        avA = av_pool.tile([65, 512], F32, tag="av", name="avA")
        avB = av_pool.tile([65, 512], F32, tag="av", name="avB")

        n_u = len(units)
        inject = {}
        for i in range(n_u):
            pt = (i + 1) * (nk - 1) // n_u if n_u else 0
            inject.setdefault(pt, []).append(units[i])

        def pop_one():
            kt, ex, off = pending.pop(0)
            for av, h in ((avA, hA), (avB, hB)):
                nc.tensor.matmul(
                    av[:, off:512],
                    lhsT=vaug_t[kt][:, h * 65:h * 65 + 65],
                    rhs=ex[:, (h & 1) * 512 + off:((h & 1) + 1) * 512],
                    start=(kt == 0), stop=(kt == nk - 1),
                    skip_group_check=True,
                )

        pending = []

        for kt in range(nk):
            j = kt - 4 * qg
            off = 128 * j if j >= 0 else 0
            diag = j >= 0
            ps = ps_pool.tile([128, 1024], F32, tag="mm", name="ps")
            for po in (0, 64):
                hf = po // 64
                nc.tensor.matmul(
                    ps[:, hf * 512 + off:(hf + 1) * 512],
                    lhsT=kT_t[ti][po:po + 64, kt * 128:(kt + 1) * 128],
                    rhs=qT_t[ti][po:po + 64, qg * 512 + off:(qg + 1) * 512],
                    start=True, stop=True,
                )
            ex = ex_pool.tile([128, 1024], BF16, tag="ex", name="ex")
            if off:
                ps_in = ps.rearrange("p (h q) -> p h q", q=512)[:, :, off:512]
                ex_out = ex.rearrange("p (h q) -> p h q", q=512)[:, :, off:512]
            else:
                ps_in, ex_out = ps, ex
            nc.scalar.activation(out=ex_out, in_=ps_in,
                                 func=mybir.ActivationFunctionType.Exp,
                                 scale=0.125)
            if diag:  # 0/1 mask on the 128-wide triangular sub-block only
                for hf in range(2):
                    exm = ex[:, hf * 512 + off:hf * 512 + off + 128]
                    nc.vector.tensor_mul(exm, exm,
                                         m01_t[:, hf * 128:(hf + 1) * 128])
            pending.append((kt, ex, off))
            # pop in PAIRS at odd kts: scores for two kts issue back-to-back
            # (cheap same-shape transitions), then two AV pairs, halving the
            # expensive score<->AV group switches; lag stays >= 2 kts
            if kt & 1:
                while len(pending) > 2:
                    pop_one()
            for u in inject.get(kt, []):
                u()

        def flush_tail():
            while pending:
                pop_one()
            _stash(hp, qg, ti, avA, avB)
        return flush_tail

    def _stash(hp, qg, ti, avA, avB):
        hA, hB = 2 * hp, 2 * hp + 1
        if qg == LAST_QG and hp == HL // 2 - 1:
            # final slot: normalize inline via reciprocal + PE broadcast
            stg2 = rec_pool.tile([64, 512], F32, tag="stg2", name="stg2")
            nc.vector.memset(stg2, 1.0)
            for av, po, row in ((avA, 0, 0), (avB, 64, 32)):
                nc.vector.tensor_copy(
                    outT_t[ti][po:po + 64, qg * 512:(qg + 1) * 512],
                    av[0:64, :])
                nc.vector.tensor_copy(stg2[row:row + 1, :], av[64:65, :])
            rec2 = rec_pool.tile([64, 512], F32, tag="rec2", name="rec2")
            nc.vector.reciprocal_approx_fast(out=rec2, in_=stg2)
            recb2 = rec_pool.tile([64, 512], BF16, tag="recb2", name="recb2")
            nc.vector.tensor_copy(recb2, rec2)
            bc = av_pool.tile([128, 512], F32, tag="av", name="bc")
            nc.tensor.matmul(bc, lhsT=ones2, rhs=recb2, start=True, stop=True)
            for po in (0, 64):
                sl = outT_t[ti][po:po + 64, qg * 512:(qg + 1) * 512]
                nc.vector.tensor_mul(sl, sl, bc[po:po + 64, :])
        else:
            for av, h, po in ((avA, hA, 0), (avB, hB, 64)):
                nc.vector.tensor_copy(
                    outT_t[ti][po:po + 64, qg * 512:(qg + 1) * 512],
                    av[0:64, :])
                stg = rec_pool.tile([1, 512], F32, tag="stg", name="stg",
                                    bufs=4)
                nc.vector.tensor_copy(stg, av[64:65, :])
                nc.sync.dma_start(out=sums_dram[qg, h], in_=stg)

    # ---- batched normalization (DRAM-bounce broadcast) -------------------
    def _norm_heads(qg, heads):
        h0, nh = heads[0], len(heads)
        sums = rec_pool.tile([nh, 512], F32, tag=f"sums{nh}", name="sums")
        nc.sync.dma_start(out=sums, in_=sums_dram[qg, h0:h0 + nh])
        rec = rec_pool.tile([nh, 512], F32, tag=f"rec{nh}", name="rec")
        nc.vector.reciprocal_approx_fast(out=rec, in_=sums)
        recb = rec_pool.tile([nh, 512], BF16, tag=f"recb{nh}", name="recb")
        nc.vector.tensor_copy(recb, rec)
        nc.sync.dma_start(out=rec_dram[qg, h0:h0 + nh], in_=recb)
        for h in heads:
            ti, po = h // 2, 64 * (h % 2)
            bcs = rec_pool.tile([128, 512], BF16, tag="bcs", name="bcs")
            nc.sync.dma_start(
                out=bcs[po:po + 64, :],
                in_=rec_dram[qg, h:h + 1, :].to_broadcast([64, 512]))
            sl = outT_t[ti][po:po + 64, qg * 512:(qg + 1) * 512]
            nc.vector.tensor_mul(sl, sl, bcs[po:po + 64, :])

    def normalize(qg):
        _norm_heads(qg, list(range(HL)))

    def normalize_pair(qg, hp):
        _norm_heads(qg, [2 * hp, 2 * hp + 1])

    # ---- o-projection: y[s,:] partial ------------------------------------
    def oproj_h(st, hf):
        ps = fl_pool.tile([128, 512], F32, tag="fl", name="fps")
        for dt_ in range(NET):
            nc.tensor.matmul(
                ps,
                lhsT=outT_t[dt_][:, st * 128:(st + 1) * 128],
                rhs=woT_t[dt_][:, hf * 512:(hf + 1) * 512],
                start=(dt_ == 0), stop=(dt_ == NET - 1),
            )
        ysb = y_pool.tile([128, 512], BF16, tag="ysb", name="ysb", bufs=4)
        nc.vector.tensor_copy(ysb, ps)
        q = nc.sync if hf == 0 else nc.gpsimd
        q.dma_start(
            out=y[st * 128:(st + 1) * 128, hf * 512:(hf + 1) * 512],
            in_=ysb)

    def oproj(st):
        oproj_h(st, 0)
        oproj_h(st, 1)

    # endgame helpers: o-projection split so the et0..2 partial sums run
    # BEFORE the final slot's inline normalization (only et3 depends on it),
    # hiding the serial reciprocal/broadcast chain behind PE work
    def oproj_part(st, hf, pool):
        ps = pool.tile([128, 512], F32,
                       tag="fl" if pool is fl_pool else "mm", name="fps")
        for dt_ in range(3):
            nc.tensor.matmul(
                ps,
                lhsT=outT_t[dt_][:, st * 128:(st + 1) * 128],
                rhs=woT_t[dt_][:, hf * 512:(hf + 1) * 512],
                start=(dt_ == 0), stop=False, skip_group_check=True,
            )
        return ps

    def oproj_fin(st, hf, ps):
        nc.tensor.matmul(
            ps,
            lhsT=outT_t[3][:, st * 128:(st + 1) * 128],
            rhs=woT_t[3][:, hf * 512:(hf + 1) * 512],
            start=False, stop=True, skip_group_check=True,
        )
        ysb = y_pool.tile([128, 512], BF16, tag="ysb", name="ysb", bufs=4)
        nc.vector.tensor_copy(ysb, ps)
        q = nc.sync if hf == 0 else nc.gpsimd
        q.dma_start(
            out=y[st * 128:(st + 1) * 128, hf * 512:(hf + 1) * 512],
            in_=ysb)

    # ---- program order ----------------------------------------------------
    def qkQ(et, scg):
        return lambda: proj_qk(wqT_t, qT_t, et, scg)

    def qkK(et, scg):
        return lambda: proj_qk(wkT_t, kT_t, et, scg)

    def V(stp):
        return lambda: proj_v(stp)

    def O(st):
        return lambda: oproj(st)

    def N_(qg):
        return lambda: normalize(qg)

    held = {}

    def OP(st, hf):
        def f():
            held[(st, hf)] = oproj_part(st, hf, fl_pool)
        return f

    proj_v(0)
    proj_v(1)
    proj_qk(wqT_t, qT_t, 0, 0)
    proj_qk(wkT_t, kT_t, 0, 0)

    # filler units injected inside each slot's kt loop (slot = (qg, hp) in
    # QG_ORDER-major, hp-minor order, s = visit index 0..15).  Deadlines:
    # qk(et,scg) before the first slot of a qg using scg that reads et;
    # v(stp) before any slot whose kt loop reaches st=2*stp; o(st) after
    # normalize of st's qg; normalize(qg) after all four qg stashes.
    fillers = {
        (0, 0): [qkQ(1, 0), qkK(1, 0)],
        (0, 1): [qkQ(2, 0), qkK(2, 0)],
        (0, 2): [qkQ(3, 0), qkK(3, 0)],
        (0, 3): [V(2), V(3)],
        (1, 0): [V(4), qkQ(0, 1), N_(0)],
        (1, 1): [V(5), qkK(0, 1)],
        (1, 2): [V(6), qkQ(1, 1)],
        (1, 3): [V(7), qkK(1, 1)],
        (3, 0): [qkQ(2, 1), qkK(2, 1), N_(1)],
        (3, 1): [qkQ(3, 1), qkK(3, 1)],
        (3, 2): [O(0), O(1), O(2)],
        (3, 3): [O(3), O(4), O(5)],
        (2, 0): [N_(3), O(6)],
        (2, 1): [O(7), O(12)],
        (2, 2): [O(13), O(14)],
        (2, 3): [O(15), OP(8, 0), OP(8, 1)],
    }
    post = {
        (2, 0): [lambda: normalize_pair(2, 0)],
        (2, 1): [lambda: normalize_pair(2, 1)],
        (2, 2): [lambda: normalize_pair(2, 2)],
    }
    for qg in QG_ORDER:
        for hp in range(HL // 2):
            flush_tail = attn(hp, qg, fillers.get((qg, hp), []))
            flush_tail()
            for f in post.get((qg, hp), []):
                f()
    # endgame: st8's et0..2 partials were prefetched in the (2,3) slot; run
    # st9's partials now (they only need np(2,0..2)) so the PE works through
    # the final slot's normalization chain, then finish et3 + drains.
    held[(9, 0)] = oproj_part(9, 0, ps_pool)
    held[(9, 1)] = oproj_part(9, 1, ps_pool)
    for st in (8, 9):
        for hf in (0, 1):
            oproj_fin(st, hf, held.pop((st, hf)))
    oproj(10)
    oproj(11)


def _build():
    nc = bacc.Bacc("TRN2", target_bir_lowering=False, debug=False,
                   num_devices=NCORES)
    xT = nc.dram_tensor("xT", [D, S], BF16, kind="ExternalInput").ap()
    wqT = nc.dram_tensor("wqT", [D, E], BF16, kind="ExternalInput").ap()
    wkT = nc.dram_tensor("wkT", [D, E], BF16, kind="ExternalInput").ap()
    wvT = nc.dram_tensor("wvT", [D, E], BF16, kind="ExternalInput").ap()
    woT = nc.dram_tensor("woT", [E, D], BF16, kind="ExternalInput").ap()
    m01 = nc.dram_tensor("m01", [128, 256], BF16, kind="ExternalInput").ap()
    y = nc.dram_tensor("y", [S, D], BF16, kind="ExternalOutput").ap()
    with tile.TileContext(nc) as tc:
        _mhsa_kernel(tc, y, xT, wqT, wkT, wvT, woT, m01)
    nc.compile()
    return nc


def get_compiled():
    global _compiled
    if _compiled is None:
        _compiled = _build()
    return _compiled


def _make_consts():
    # m01[k, qq] = 1 iff query qq >= key k within the 128-wide diagonal
    # sub-block; duplicated for the two packed heads.
    tri = np.triu(np.ones((128, 128), dtype=np.float32))
    m01 = np.concatenate([tri, tri], axis=1)
    return m01.astype(bf16)


def kernel(**inputs):
    global last_results
    x = np.asarray(inputs["in_features"], dtype=np.float32)
    w_q = np.asarray(inputs["w_q"], dtype=np.float32)
    w_k = np.asarray(inputs["w_k"], dtype=np.float32)
    w_v = np.asarray(inputs["w_v"], dtype=np.float32)
    w_o = np.asarray(inputs["w_o"], dtype=np.float32)

    nc = get_compiled()
    m01 = _make_consts()
    in_maps = []
    for c in range(NCORES):
        b, hg = divmod(c, 2)
        es = slice(hg * E, (hg + 1) * E)
        in_maps.append({
            "xT": x[b].T.astype(bf16),
            "wqT": w_q[es, :].T.astype(bf16),
            "wkT": w_k[es, :].T.astype(bf16),
            "wvT": w_v[es, :].T.astype(bf16),
            "woT": w_o[:, es].T.astype(bf16),
            "m01": m01,
        })
    res = run_bass_kernel_spmd(nc, in_maps, list(range(NCORES)))
    last_results = res
    y = np.zeros((B, S, D), dtype=np.float32)
    for c in range(NCORES):
        y[c // 2] += np.asarray(res.results[c]["y"], dtype=np.float32)
    return y
